# revision 9
# baseline (speedup 1.0000x reference)
"""Trainium2 Bass kernel for nn_Block_6012954214590.

Pipeline (per batch element, data-parallel over 8 NeuronCores):
  maxpool2x2 -> conv7x7+BN+ReLU -> conv3x3+BN+ReLU -> conv1x1+BN+ReLU
  -> Mamba block (in_proj [+folded depthwise causal conv], silu, x_proj,
     softplus dt, selective scan over L=4096 via DVE tensor_tensor_scan,
     gate, out_proj) -> residual -> conv1x1+BN+ReLU.
BN uses global batch statistics via tiny cross-core AllReduces (64x2 f32).
p1/p2 outputs are LayerNorm over a singleton channel axis == lnb exactly.
"""
import numpy as np
import ml_dtypes

import concourse.bass as bass
import concourse.bacc as bacc
import concourse.mybir as mybir
import concourse.tile as tile
from concourse import bass_utils

N_CORES = 8
C = 64          # conv channels
H = 64          # post-pool spatial
W = 64
L = H * W       # 4096
DI = 128        # mamba inner dim
DS = 16         # mamba state dim
R = 4           # dt rank
EPS = 1e-5
NT = 8          # N-tiles of 512 over L
TS = 512
f32 = mybir.dt.float32
bf16 = mybir.dt.bfloat16
AF = mybir.ActivationFunctionType
OP = mybir.AluOpType
bfnp = ml_dtypes.bfloat16

_cache = {}


def _bn_block(nc, pool, dram, psum_src, gcol, bcol, n_local, n_global, tag):
    """Compute global-batch BN scale/bias from a psum tensor (64, L).

    Returns (s_col, t_col) f32 (64,1) tiles: out = relu(s*y + t).
    """
    stats6 = pool.tile([C, 8 * 6], f32, name=f"stats6_{tag}")
    src3d = psum_src.rearrange("p (n f) -> p n f", f=TS)
    for i in range(8):
        nc.vector.bn_stats(stats6[:, i * 6:(i + 1) * 6], src3d[:, i, :])
    mv = pool.tile([C, 2], f32, name=f"mv_{tag}")
    nc.vector.bn_aggr(mv[:], stats6[:].rearrange("p (n s) -> p n s", s=6))
    # pack per-core (sum, sumsq)
    packed = pool.tile([C, 2], f32, name=f"packed_{tag}")
    nc.vector.tensor_scalar_mul(packed[:, 0:1], mv[:, 0:1], float(n_local))
    m2 = pool.tile([C, 1], f32, name=f"m2_{tag}")
    nc.vector.tensor_tensor(m2[:], mv[:, 0:1], mv[:, 0:1], OP.mult)
    vp = pool.tile([C, 1], f32, name=f"vp_{tag}")
    nc.vector.tensor_tensor(vp[:], mv[:, 1:2], m2[:], OP.add)
    nc.vector.tensor_scalar_mul(packed[:, 1:2], vp[:], float(n_local))
    # allreduce
    cin = dram.tile([C, 2], f32, name=f"arin_{tag}")
    cout = dram.tile([C, 2], f32, name=f"arout_{tag}")
    nc.sync.dma_start(cin[:], packed[:])
    nc.gpsimd.collective_compute(
        "AllReduce", OP.add, replica_groups=[list(range(N_CORES))],
        ins=[cin.opt()], outs=[cout.opt()],
    )
    glob = pool.tile([C, 2], f32, name=f"glob_{tag}")
    nc.sync.dma_start(glob[:], cout[:])
    # mu, var, rsqrt
    mu = pool.tile([C, 1], f32, name=f"mu_{tag}")
    nc.vector.tensor_scalar_mul(mu[:], glob[:, 0:1], 1.0 / n_global)
    e2 = pool.tile([C, 1], f32, name=f"e2_{tag}")
    nc.vector.tensor_scalar_mul(e2[:], glob[:, 1:2], 1.0 / n_global)
    m2g = pool.tile([C, 1], f32, name=f"m2g_{tag}")
    nc.vector.tensor_tensor(m2g[:], mu[:], mu[:], OP.mult)
    v = pool.tile([C, 1], f32, name=f"v_{tag}")
    nc.vector.tensor_tensor(v[:], e2[:], m2g[:], OP.subtract)
    nc.vector.tensor_scalar_add(v[:], v[:], EPS)
    lnv = pool.tile([C, 1], f32, name=f"lnv_{tag}")
    nc.scalar.activation(lnv[:], v[:], AF.Ln)
    rs = pool.tile([C, 1], f32, name=f"rs_{tag}")
    nc.scalar.activation(rs[:], lnv[:], AF.Exp, scale=-0.5)
    s_col = pool.tile([C, 1], f32, name=f"scol_{tag}")
    nc.vector.tensor_tensor(s_col[:], rs[:], gcol, OP.mult)
    ms = pool.tile([C, 1], f32, name=f"ms_{tag}")
    nc.vector.tensor_tensor(ms[:], mu[:], s_col[:], OP.mult)
    t_col = pool.tile([C, 1], f32, name=f"tcol_{tag}")
    nc.vector.tensor_tensor(t_col[:], bcol, ms[:], OP.subtract)
    return s_col, t_col


def build_program():
    nc = bacc.Bacc("TRN2", target_bir_lowering=False, debug=False,
                   enable_asserts=False, num_devices=N_CORES)

    def din(name, shape, dt):
        return nc.dram_tensor(name, shape, dt, kind="ExternalInput").ap()

    ximg = din("ximg", [3, 128, 128], f32)
    wc1a = din("wc1a", [63, C], bf16)   # rows (khl, kw, ci) for kh 0-2
    wc1b = din("wc1b", [63, C], bf16)   # kh 3-5
    wc1c = din("wc1c", [21, C], bf16)   # kh 6
    wc2 = din("wc2", [64, 9 * C], bf16)  # [ci, (tap co)]
    wc3 = din("wc3", [64, 2 * C], bf16)  # duplicated output channels
    wina = din("wina", [128, DI], bf16)  # (j*64+c, d) taps 0,1 of folded dwconv
    winb = din("winb", [128, DI], bf16)  # taps 2,3
    wz = din("wz", [64, DI], bf16)
    wdt = din("wdt", [DI, DI], bf16)     # dt_w @ x_proj[:4] transposed
    wbc = din("wbc", [DI, 2 * DS], bf16)
    wop = din("wop", [DI, C], bf16)
    wc4 = din("wc4", [64, C], bf16)
    convb = din("convb", [DI, 1], f32)
    dtb = din("dtb", [DI, 1], f32)
    acols = din("acols", [DI, DS], f32)  # A = -exp(A_log)
    dcol = din("dcol", [DI, 1], f32)
    gbcols = din("gbcols", [C, 8], f32)  # g1 b1 g2 b2 g3 b3 g4 b4
    out_y = nc.dram_tensor("out_y", [C, L], f32, kind="ExternalOutput").ap()

    with tile.TileContext(nc) as tc:
        with tc.tile_pool(name="persist", bufs=1) as pp, \
             tc.tile_pool(name="dram", bufs=1, space="DRAM") as dram:
            gb = pp.tile([C, 8], f32, name="gb")
            nc.sync.dma_start(gb[:], gbcols[:])
            acol_t = pp.tile([DI, DS], f32, name="acol_t")
            nc.sync.dma_start(acol_t[:], acols[:])
            convb_t = pp.tile([DI, 1], f32, name="convb_t")
            nc.sync.dma_start(convb_t[:], convb[:])
            dtb_t = pp.tile([DI, 1], f32, name="dtb_t")
            nc.sync.dma_start(dtb_t[:], dtb[:])
            dcol_t = pp.tile([DI, 1], f32, name="dcol_t")
            nc.sync.dma_start(dcol_t[:], dcol[:])
            ones1 = pp.tile([65, 128], bf16, name="ones1")
            nc.vector.memset(ones1[:], 1.0)
            # weights
            wc2_t = pp.tile([64, 9 * C], bf16, name="wc2_t")
            nc.sync.dma_start(wc2_t[:], wc2[:])
            wc3_t = pp.tile([64, 2 * C], bf16, name="wc3_t")
            nc.sync.dma_start(wc3_t[:], wc3[:])
            wina_t = pp.tile([128, DI], bf16, name="wina_t")
            nc.sync.dma_start(wina_t[:], wina[:])
            winb_t = pp.tile([128, DI], bf16, name="winb_t")
            nc.sync.dma_start(winb_t[:], winb[:])
            wz_t = pp.tile([64, DI], bf16, name="wz_t")
            nc.sync.dma_start(wz_t[:], wz[:])
            wdt_t = pp.tile([DI, DI], bf16, name="wdt_t")
            nc.sync.dma_start(wdt_t[:], wdt[:])
            wbc_t = pp.tile([DI, 2 * DS], bf16, name="wbc_t")
            nc.sync.dma_start(wbc_t[:], wbc[:])
            wop_t = pp.tile([DI, C], bf16, name="wop_t")
            nc.sync.dma_start(wop_t[:], wop[:])
            wc4_t = pp.tile([64, C], bf16, name="wc4_t")
            nc.sync.dma_start(wc4_t[:], wc4[:])

            # ---- Phase 1: maxpool + conv1 + BN1 -> x2pad ----
            cpool = tc.tile_pool(name="convs", bufs=1)
            cp_ = cpool.__enter__()
            x2pad = cp_.tile([64, 66 * 66], bf16, name="x2pad")
            nc.vector.memset(x2pad[:], 0.0)
            with tc.tile_pool(name="ph1", bufs=1) as p1pool, \
                 tc.tile_pool(name="ph1psum", bufs=1, space="PSUM") as psum1:
                T = p1pool.tile([64, 768], f32, name="T")
                nc.sync.dma_start(
                    T[:].rearrange("p (c hp w) -> p c hp w", c=3, hp=2),
                    ximg.rearrange("c (p hp) w -> p c hp w", hp=2))
                T4 = T[:].rearrange("p (c hp w) -> p c hp w", c=3, hp=2)
                P2 = p1pool.tile([64, 384], f32, name="P2")
                P24 = P2[:].rearrange("p (c hp w) -> p c hp w", c=3, hp=2)
                nc.vector.tensor_tensor(P24, T4[:, :, :, 0:128:2],
                                        T4[:, :, :, 1:128:2], OP.max)
                Pool = p1pool.tile([64, 192], f32, name="Pool")
                P23 = P2[:].rearrange("p (c hp w) -> p c hp w", c=3, hp=2)
                Pl3 = Pool[:].rearrange("p (c w) -> p c w", c=3)
                nc.vector.tensor_tensor(Pl3, P23[:, :, 0, :], P23[:, :, 1, :],
                                        OP.max)
                # stage pooled to DRAM as bf16 (3, 64, 64)
                dpool = dram.tile([3, 64, 64], bf16, name="dpool")
                nc.gpsimd.dma_start(dpool[:].rearrange("c p w -> p c w"), Pl3)
                # X1: partitions (khl, kw, ci); content xpadflat shifted by
                # khl*70 + kw
                X1 = p1pool.tile([63, 4900], bf16, name="X1")
                nc.vector.memset(X1[:], 0.0)
                for khl in range(3):
                    for kw in range(7):
                        p0 = khl * 21 + kw * 3
                        o = (3 - khl) * 70 + (3 - kw)
                        dst = X1[p0:p0 + 3, o:o + 4480]
                        dst3 = dst.rearrange("p (r c) -> p r c", c=70)
                        nc.sync.dma_start(dst3[:, 0:64, 0:64], dpool[:])
                psum_y1 = psum1.tile([C, L], f32, name="psum_y1")
                X1r = X1[:].rearrange("p (r c) -> p r c", c=70)
                y1r = psum_y1[:].rearrange("p (r c) -> p r c", c=64)
                wc1a_t = p1pool.tile([63, C], bf16, name="wc1a_t")
                nc.sync.dma_start(wc1a_t[:], wc1a[:])
                wc1b_t = p1pool.tile([63, C], bf16, name="wc1b_t")
                nc.sync.dma_start(wc1b_t[:], wc1b[:])
                wc1c_t = p1pool.tile([21, C], bf16, name="wc1c_t")
                nc.sync.dma_start(wc1c_t[:], wc1c[:])
                for th in range(NT):
                    r0 = th * 8
                    nc.tensor.matmul(y1r[:, r0:r0 + 8, :],
                                     wc1a_t[:], X1r[:, r0:r0 + 8, 0:64],
                                     start=True, stop=False)
                    nc.tensor.matmul(y1r[:, r0:r0 + 8, :],
                                     wc1b_t[:], X1r[:, 3 + r0:3 + r0 + 8, 0:64],
                                     start=False, stop=False)
                    nc.tensor.matmul(y1r[:, r0:r0 + 8, :],
                                     wc1c_t[0:21, :],
                                     X1r[0:21, 6 + r0:6 + r0 + 8, 0:64],
                                     start=False, stop=True)
                s1, t1 = _bn_block(nc, p1pool, dram, psum_y1[:],
                                   gb[:, 0:1], gb[:, 1:2], L, N_CORES * L, "bn1")
                x2r = x2pad[:].rearrange("p (r c) -> p r c", c=66)
                for th in range(NT):
                    nc.scalar.activation(
                        x2r[:, 1 + th * 8:1 + th * 8 + 8, 1:65],
                        y1r[:, th * 8:th * 8 + 8, :],
                        AF.Relu, bias=t1[:], scale=s1[:])

            # ---- Phase 2: conv2 + BN2 -> xb2 ----
            xb2 = cp_.tile([64, L], bf16, name="xb2")
            with tc.tile_pool(name="ph2", bufs=1) as p2pool, \
                 tc.tile_pool(name="ph2psum", bufs=1, space="PSUM") as psum2:
                psum_y2 = psum2.tile([C, L], f32, name="psum_y2")
                y2r = psum_y2[:].rearrange("p (r c) -> p r c", c=64)
                x2r = x2pad[:].rearrange("p (r c) -> p r c", c=66)
                for th in range(NT):
                    r0 = th * 8
                    first = True
                    for kh in range(3):
                        for kw in range(3):
                            nc.tensor.matmul(
                                y2r[:, r0:r0 + 8, :],
                                wc2_t[:, (kh * 3 + kw) * C:(kh * 3 + kw + 1) * C],
                                x2r[:, kh + r0:kh + r0 + 8, kw:kw + 64],
                                start=first, stop=(kh == 2 and kw == 2))
                            first = False
                s2, t2 = _bn_block(nc, p2pool, dram, psum_y2[:],
                                   gb[:, 2:3], gb[:, 3:4], L, N_CORES * L, "bn2")
                nc.scalar.activation(xb2[:], psum_y2[:], AF.Relu,
                                     bias=t2[:], scale=s2[:])

            # ---- Phase 3: conv3 (dup M=128) + BN3 -> X3s ----
            X3s = pp.tile([128, L + 4], bf16, name="X3s")
            nc.vector.memset(X3s[:], 0.0)
            with tc.tile_pool(name="ph3", bufs=1) as p3pool, \
                 tc.tile_pool(name="ph3psum", bufs=1, space="PSUM") as psum3:
                psum_y3 = psum3.tile([128, L], f32, name="psum_y3")
                for th in range(NT):
                    nc.tensor.matmul(psum_y3[:, th * TS:(th + 1) * TS],
                                     wc3_t[:], xb2[:, th * TS:(th + 1) * TS],
                                     start=True, stop=True)
                s3, t3 = _bn_block(nc, p3pool, dram, psum_y3[0:C, :],
                                   gb[:, 4:5], gb[:, 5:6], L, N_CORES * L, "bn3")
                nc.scalar.activation(X3s[0:64, 3:3 + L], psum_y3[0:64, :],
                                     AF.Relu, bias=t3[:], scale=s3[:])
                nc.scalar.activation(X3s[64:128, 2:2 + L], psum_y3[64:128, :],
                                     AF.Relu, bias=t3[:], scale=s3[:])
            cpool.__exit__(None, None, None)

            # ---- Phase 4: mamba projections ----
            xs_bf = pp.tile([DI, L], bf16, name="xs_bf")
            g_bf = pp.tile([DI, L], bf16, name="g_bf")
            dt_f = pp.tile([DI, L], f32, name="dt_f")
            u_bf = pp.tile([DI, L], bf16, name="u_bf")
            bc_bf = pp.tile([2 * DS, L], bf16, name="bc_bf")
            silu_insts = []
            with tc.tile_pool(name="ph4psum", bufs=2, space="PSUM") as psum4, \
                 tc.tile_pool(name="ph4", bufs=2) as p4pool:
                for th in range(NT):
                    sl = slice(th * TS, (th + 1) * TS)
                    psum_xc = psum4.tile([DI, TS], f32, tag="psum_xc")
                    nc.tensor.matmul(psum_xc[:], wina_t[:],
                                     X3s[:, th * TS:th * TS + TS],
                                     start=True, stop=False)
                    nc.tensor.matmul(psum_xc[:], winb_t[:],
                                     X3s[:, th * TS + 2:th * TS + 2 + TS],
                                     start=False, stop=True)
                    i1 = nc.scalar.activation(xs_bf[:, sl], psum_xc[:],
                                              AF.Silu, bias=convb_t[:])
                    psum_z = psum4.tile([DI, TS], f32, tag="psum_z")
                    nc.tensor.matmul(psum_z[:], wz_t[:],
                                     X3s[0:64, th * TS + 3:th * TS + 3 + TS],
                                     start=True, stop=True)
                    i2 = nc.scalar.activation(g_bf[:, sl], psum_z[:], AF.Silu)
                    silu_insts.append(i1)
                    silu_insts.append(i2)
                for th in range(NT):
                    sl = slice(th * TS, (th + 1) * TS)
                    psum_dt = psum4.tile([DI, TS], f32, tag="psum_dt")
                    nc.tensor.matmul(psum_dt[:], wdt_t[:], xs_bf[:, sl],
                                     start=True, stop=True)
                    et = p4pool.tile([DI, TS], f32, tag="et")
                    e1 = nc.scalar.activation(et[:], psum_dt[:], AF.Exp,
                                              bias=dtb_t[:])
                    nc.scalar.activation(dt_f[:, sl], et[:], AF.Ln, bias=1.0)
                    if th == 0:
                        for si in silu_insts:
                            tile.add_dep_helper(e1.ins, si.ins, sync=False,
                                                reason="act table grouping")
                    psum_bc = psum4.tile([2 * DS, TS], f32, tag="psum_bc")
                    nc.tensor.matmul(psum_bc[:], wbc_t[:], xs_bf[:, sl],
                                     start=True, stop=True)
                    nc.scalar.activation(bc_bf[:, sl], psum_bc[:], AF.Copy)
                    nc.vector.tensor_tensor(u_bf[:, sl], dt_f[:, sl],
                                            xs_bf[:, sl], OP.mult)

            # ---- Phase 5: selective scan over s=0..15 ----
            y0 = pp.tile([DI, L], bf16, name="y0")
            y1t = pp.tile([DI, L], bf16, name="y1t")
            with tc.tile_pool(name="ph5psum", bufs=2, space="PSUM") as psum5, \
                 tc.tile_pool(name="ph5", bufs=2) as p5pool:
                for s in range(DS):
                    stage = p5pool.tile([65, L], bf16, tag="stage", bufs=2)
                    nc.sync.dma_start(stage[0:1, :], bc_bf[s:s + 1, :])
                    nc.sync.dma_start(stage[64:65, :],
                                      bc_bf[DS + s:DS + s + 1, :])
                    brep = p5pool.tile([DI, L], bf16, tag="brep", bufs=2)
                    crep = p5pool.tile([DI, L], bf16, tag="crep", bufs=2)
                    for half in range(2):
                        hs = slice(half * 2048, (half + 1) * 2048)
                        prep = psum5.tile([DI, 2048], f32, tag="prep", bufs=2)
                        for q in range(4):
                            qs = slice(half * 2048 + q * TS,
                                       half * 2048 + (q + 1) * TS)
                            nc.tensor.matmul(prep[:, q * TS:(q + 1) * TS],
                                             ones1[0:1, :], stage[0:1, qs],
                                             start=True, stop=True)
                        nc.scalar.activation(brep[:, hs], prep[:], AF.Copy)
                        prep2 = psum5.tile([DI, 2048], f32, tag="prep", bufs=2)
                        for q in range(4):
                            qs = slice(half * 2048 + q * TS,
                                       half * 2048 + (q + 1) * TS)
                            nc.tensor.matmul(prep2[:, q * TS:(q + 1) * TS],
                                             ones1[64:65, :], stage[64:65, qs],
                                             start=True, stop=True)
                        nc.scalar.activation(crep[:, hs], prep2[:], AF.Copy)
                    a_bf = p5pool.tile([DI, L], bf16, tag="a_bf", bufs=2)
                    nc.scalar.activation(a_bf[:], dt_f[:], AF.Exp,
                                         scale=acol_t[:, s:s + 1])
                    b_bf = p5pool.tile([DI, L], bf16, tag="b_bf", bufs=2)
                    h_bf = p5pool.tile([DI, L], bf16, tag="h_bf", bufs=2)
                    p_bf = p5pool.tile([DI, L], bf16, tag="p_bf", bufs=2)
                    yacc = y0 if s % 2 == 0 else y1t
                    if s % 2 == 0:
                        nc.vector.tensor_tensor(b_bf[:], u_bf[:], brep[:], OP.mult)
                        nc.vector.tensor_tensor_scan(h_bf[:], a_bf[:], b_bf[:],
                                                     0.0, OP.mult, OP.add)
                        nc.gpsimd.tensor_tensor(p_bf[:], h_bf[:], crep[:], OP.mult)
                        if s < 2:
                            nc.vector.tensor_copy(yacc[:], p_bf[:])
                        else:
                            nc.vector.tensor_tensor(yacc[:], yacc[:], p_bf[:],
                                                    OP.add)
                    else:
                        nc.gpsimd.tensor_tensor(b_bf[:], u_bf[:], brep[:], OP.mult)
                        nc.vector.tensor_tensor_scan(h_bf[:], a_bf[:], b_bf[:],
                                                     0.0, OP.mult, OP.add)
                        nc.vector.tensor_tensor(p_bf[:], h_bf[:], crep[:], OP.mult)
                        if s < 2:
                            nc.gpsimd.tensor_copy(yacc[:], p_bf[:])
                        else:
                            nc.gpsimd.tensor_tensor(yacc[:], yacc[:], p_bf[:],
                                                    OP.add)

            # ---- Phase 6: tail ----
            with tc.tile_pool(name="ph6", bufs=1) as p6pool:
                out_f = p6pool.tile([C, L], f32, name="out_f")
                ysum = p6pool.tile([DI, L], bf16, name="ysum")
                nc.vector.tensor_tensor(ysum[:], y0[:], y1t[:], OP.add)
                dx = p6pool.tile([DI, L], bf16, name="dx")
                nc.vector.tensor_scalar_mul(dx[:], xs_bf[:], dcol_t[:])
                y2m = p6pool.tile([DI, L], bf16, name="y2m")
                nc.vector.tensor_tensor(y2m[:], ysum[:], dx[:], OP.add)
                y3m = p6pool.tile([DI, L], bf16, name="y3m")
                nc.vector.tensor_tensor(y3m[:], y2m[:], g_bf[:], OP.mult)
                m_bf = p6pool.tile([C, L], bf16, name="m_bf")
                with tc.tile_pool(name="ph6psum", bufs=2,
                                  space="PSUM") as psum6:
                    for th in range(NT):
                        sl = slice(th * TS, (th + 1) * TS)
                        psum_m = psum6.tile([C, TS], f32, tag="psum_m")
                        nc.tensor.matmul(psum_m[:], wop_t[:], y3m[:, sl],
                                         start=True, stop=True)
                        nc.scalar.activation(m_bf[:, sl], psum_m[:], AF.Copy)
                with tc.tile_pool(name="ph6psum2", bufs=1,
                                  space="PSUM") as psum7:
                    psum_y4 = psum7.tile([C, L], f32, name="psum_y4")
                    for th in range(NT):
                        sl = slice(th * TS, (th + 1) * TS)
                        nc.tensor.matmul(psum_y4[:, sl], wc4_t[:], m_bf[:, sl],
                                         start=True, stop=False)
                        nc.tensor.matmul(psum_y4[:, sl], wc4_t[:],
                                         X3s[0:64, 3 + th * TS:3 + th * TS + TS],
                                         start=False, stop=True)
                    s4, t4 = _bn_block(nc, p6pool, dram, psum_y4[:],
                                       gb[:, 6:7], gb[:, 7:8], L,
                                       N_CORES * L, "bn4")
                    nc.scalar.activation(out_f[:], psum_y4[:], AF.Relu,
                                         bias=t4[:], scale=s4[:])
            nc.sync.dma_start(out_y[:], out_f[:])

    nc.compile()
    return nc


def _prep_weights(I):
    """Host-side weight layout prep (all tiny)."""
    bf = lambda a: np.ascontiguousarray(a, dtype=np.float32).astype(bfnp)
    f = lambda a: np.ascontiguousarray(a, dtype=np.float32)
    w1 = np.asarray(I["w1"], np.float32)      # (64, 3, 7, 7)
    t1 = w1.transpose(2, 3, 1, 0)             # (kh, kw, ci, co)
    # partitions (khl, kw, ci)
    wc1a = t1[0:3].transpose(0, 1, 2, 3).reshape(3, 7, 3, C)
    mk1 = lambda g: np.ascontiguousarray(
        t1[g * 3:g * 3 + 3].reshape(3, 7, 3, C)).reshape(63, C)
    wc1a = mk1(0)
    wc1b = mk1(1)
    wc1c = np.ascontiguousarray(t1[6:7].reshape(1, 7, 3, C)).reshape(21, C)
    w2 = np.asarray(I["w2"], np.float32)
    wc2 = w2.transpose(2, 3, 1, 0).reshape(9, C, C)  # (tap, ci, co)
    wc2 = np.ascontiguousarray(wc2.transpose(1, 0, 2)).reshape(C, 9 * C)
    w3 = np.asarray(I["w3"], np.float32)[:, :, 0, 0]  # (co, ci)
    wc3 = np.concatenate([w3.T, w3.T], axis=1)        # (ci, 2*co)
    in_proj = np.asarray(I["in_proj"], np.float32)    # (256, 64)
    Wxs, Wz = in_proj[:DI], in_proj[DI:]
    cw = np.asarray(I["conv_w"], np.float32)[:, 0, :]  # (di, 4)
    # wina[j*64+c, d] = cw[d, j] * Wxs[d, c], j in {0,1}
    wina = np.empty((128, DI), np.float32)
    winb = np.empty((128, DI), np.float32)
    for j in range(2):
        wina[j * 64:(j + 1) * 64] = (cw[:, j][:, None] * Wxs).T
        winb[j * 64:(j + 1) * 64] = (cw[:, 2 + j][:, None] * Wxs).T
    x_proj = np.asarray(I["x_proj"], np.float32)       # (36, 128)
    dt_pre = np.asarray(I["dt_w"], np.float32) @ x_proj[:R]  # (di, di)
    wdt = dt_pre.T
    wbc = x_proj[R:].T                                  # (128, 32)
    wop = np.asarray(I["out_proj"], np.float32).T       # (128, 64)
    w4 = np.asarray(I["w4"], np.float32)[:, :, 0, 0]
    wc4 = w4.T
    A = -np.exp(np.asarray(I["A_log"], np.float32))     # (di, ds)
    gbcols = np.stack([f(I["g1"]), f(I["b1"]), f(I["g2"]), f(I["b2"]),
                       f(I["g3"]), f(I["b3"]), f(I["g4"]), f(I["b4"])],
                      axis=1)
    return {
        "wc1a": bf(wc1a), "wc1b": bf(wc1b), "wc1c": bf(wc1c),
        "wc2": bf(wc2), "wc3": bf(wc3),
        "wina": bf(wina), "winb": bf(winb), "wz": bf(Wz.T),
        "wdt": bf(wdt), "wbc": bf(wbc), "wop": bf(wop), "wc4": bf(wc4),
        "convb": f(I["conv_b"]).reshape(DI, 1),
        "dtb": f(I["dt_b"]).reshape(DI, 1),
        "acols": f(A), "dcol": f(I["D"]).reshape(DI, 1),
        "gbcols": f(gbcols),
    }


def kernel(**inputs):
    if "nc" not in _cache:
        _cache["nc"] = build_program()
    nc = _cache["nc"]
    wmap = _prep_weights(inputs)
    x = np.asarray(inputs["x"], np.float32)  # (8, 3, 128, 128)
    in_maps = []
    for b in range(N_CORES):
        m = dict(wmap)
        m["ximg"] = np.ascontiguousarray(x[b])
        in_maps.append(m)
    import os
    trace = bool(os.environ.get("KERNEL_TRACE"))
    if trace:
        try:
            import trace_shim  # noqa: F401  (dev-only profiling hook)
        except ImportError:
            trace = False
    res = bass_utils.run_bass_kernel_spmd(nc, in_maps,
                                          core_ids=list(range(N_CORES)),
                                          trace=trace)
    _cache["exec_time_ns"] = res.exec_time_ns
    out = np.empty((8, C, H, W), np.float32)
    for b in range(N_CORES):
        out[b] = res.results[b]["out_y"].reshape(C, H, W)
    lnb1 = np.asarray(inputs["lnb1"], np.float32).reshape(1, 1, 1, 1)
    lnb2 = np.asarray(inputs["lnb2"], np.float32).reshape(1, 1, 1, 1)
    p1 = np.broadcast_to(lnb1, (8, 1, H, W)).copy()
    p2 = np.broadcast_to(lnb2, (8, 1, H, W)).copy()
    return out, p1, p2


# revision 10
# speedup vs baseline: 1.0044x; 1.0044x over previous
"""Trainium2 Bass kernel for nn_Block_6012954214590.

Pipeline (per batch element, data-parallel over 8 NeuronCores):
  maxpool2x2 -> conv7x7+BN+ReLU -> conv3x3+BN+ReLU -> conv1x1+BN+ReLU
  -> Mamba block (in_proj [+folded depthwise causal conv], silu, x_proj,
     softplus dt, selective scan over L=4096 via DVE tensor_tensor_scan,
     gate, out_proj) -> residual -> conv1x1+BN+ReLU.
BN uses global batch statistics via tiny cross-core AllReduces (64x2 f32).
p1/p2 outputs are LayerNorm over a singleton channel axis == lnb exactly.
"""
import numpy as np
import ml_dtypes

import concourse.bass as bass
import concourse.bacc as bacc
import concourse.mybir as mybir
import concourse.tile as tile
from concourse import bass_utils

N_CORES = 8
C = 64          # conv channels
H = 64          # post-pool spatial
W = 64
L = H * W       # 4096
DI = 128        # mamba inner dim
DS = 16         # mamba state dim
R = 4           # dt rank
EPS = 1e-5
NT = 8          # N-tiles of 512 over L
TS = 512
f32 = mybir.dt.float32
bf16 = mybir.dt.bfloat16
AF = mybir.ActivationFunctionType
OP = mybir.AluOpType
bfnp = ml_dtypes.bfloat16

_cache = {}


def _bn_block(nc, pool, dram, psum_src, gcol, bcol, n_local, n_global, tag):
    """Compute global-batch BN scale/bias from a psum tensor (64, L).

    Returns (s_col, t_col) f32 (64,1) tiles: out = relu(s*y + t).
    """
    stats6 = pool.tile([C, 8 * 6], f32, name=f"stats6_{tag}")
    src3d = psum_src.rearrange("p (n f) -> p n f", f=TS)
    for i in range(8):
        nc.vector.bn_stats(stats6[:, i * 6:(i + 1) * 6], src3d[:, i, :])
    mv = pool.tile([C, 2], f32, name=f"mv_{tag}")
    nc.vector.bn_aggr(mv[:], stats6[:].rearrange("p (n s) -> p n s", s=6))
    # pack per-core (sum, sumsq)
    packed = pool.tile([C, 2], f32, name=f"packed_{tag}")
    nc.vector.tensor_scalar_mul(packed[:, 0:1], mv[:, 0:1], float(n_local))
    m2 = pool.tile([C, 1], f32, name=f"m2_{tag}")
    nc.vector.tensor_tensor(m2[:], mv[:, 0:1], mv[:, 0:1], OP.mult)
    vp = pool.tile([C, 1], f32, name=f"vp_{tag}")
    nc.vector.tensor_tensor(vp[:], mv[:, 1:2], m2[:], OP.add)
    nc.vector.tensor_scalar_mul(packed[:, 1:2], vp[:], float(n_local))
    # allreduce
    cin = dram.tile([C, 2], f32, name=f"arin_{tag}")
    cout = dram.tile([C, 2], f32, name=f"arout_{tag}")
    nc.sync.dma_start(cin[:], packed[:])
    nc.gpsimd.collective_compute(
        "AllReduce", OP.add, replica_groups=[list(range(N_CORES))],
        ins=[cin.opt()], outs=[cout.opt()],
    )
    glob = pool.tile([C, 2], f32, name=f"glob_{tag}")
    nc.sync.dma_start(glob[:], cout[:])
    # mu, var, rsqrt
    mu = pool.tile([C, 1], f32, name=f"mu_{tag}")
    nc.vector.tensor_scalar_mul(mu[:], glob[:, 0:1], 1.0 / n_global)
    e2 = pool.tile([C, 1], f32, name=f"e2_{tag}")
    nc.vector.tensor_scalar_mul(e2[:], glob[:, 1:2], 1.0 / n_global)
    m2g = pool.tile([C, 1], f32, name=f"m2g_{tag}")
    nc.vector.tensor_tensor(m2g[:], mu[:], mu[:], OP.mult)
    v = pool.tile([C, 1], f32, name=f"v_{tag}")
    nc.vector.tensor_tensor(v[:], e2[:], m2g[:], OP.subtract)
    nc.vector.tensor_scalar_add(v[:], v[:], EPS)
    lnv = pool.tile([C, 1], f32, name=f"lnv_{tag}")
    nc.scalar.activation(lnv[:], v[:], AF.Ln)
    rs = pool.tile([C, 1], f32, name=f"rs_{tag}")
    nc.scalar.activation(rs[:], lnv[:], AF.Exp, scale=-0.5)
    s_col = pool.tile([C, 1], f32, name=f"scol_{tag}")
    nc.vector.tensor_tensor(s_col[:], rs[:], gcol, OP.mult)
    ms = pool.tile([C, 1], f32, name=f"ms_{tag}")
    nc.vector.tensor_tensor(ms[:], mu[:], s_col[:], OP.mult)
    t_col = pool.tile([C, 1], f32, name=f"tcol_{tag}")
    nc.vector.tensor_tensor(t_col[:], bcol, ms[:], OP.subtract)
    return s_col, t_col


def build_program():
    nc = bacc.Bacc("TRN2", target_bir_lowering=False, debug=False,
                   enable_asserts=False, num_devices=N_CORES)

    def din(name, shape, dt):
        return nc.dram_tensor(name, shape, dt, kind="ExternalInput").ap()

    ximg = din("ximg", [3, 128, 128], f32)
    wc1a = din("wc1a", [63, C], bf16)   # rows (khl, kw, ci) for kh 0-2
    wc1b = din("wc1b", [63, C], bf16)   # kh 3-5
    wc1c = din("wc1c", [21, C], bf16)   # kh 6
    wc2 = din("wc2", [64, 9 * C], bf16)  # [ci, (tap co)]
    wc3 = din("wc3", [64, 2 * C], bf16)  # duplicated output channels
    wina = din("wina", [128, DI], bf16)  # (j*64+c, d) taps 0,1 of folded dwconv
    winb = din("winb", [128, DI], bf16)  # taps 2,3
    wz = din("wz", [64, DI], bf16)
    wdt = din("wdt", [DI, DI], bf16)     # dt_w @ x_proj[:4] transposed
    wbc = din("wbc", [DI, 2 * DS], bf16)
    wop = din("wop", [DI, C], bf16)
    wc4 = din("wc4", [64, C], bf16)
    convb = din("convb", [DI, 1], f32)
    dtb = din("dtb", [DI, 1], f32)
    acols = din("acols", [DI, DS], f32)  # A = -exp(A_log)
    dcol = din("dcol", [DI, 1], f32)
    gbcols = din("gbcols", [C, 8], f32)  # g1 b1 g2 b2 g3 b3 g4 b4
    out_y = nc.dram_tensor("out_y", [C, L], f32, kind="ExternalOutput").ap()

    with tile.TileContext(nc) as tc:
        with tc.tile_pool(name="persist", bufs=1) as pp, \
             tc.tile_pool(name="dram", bufs=1, space="DRAM") as dram:
            gb = pp.tile([C, 8], f32, name="gb")
            nc.sync.dma_start(gb[:], gbcols[:])
            acol_t = pp.tile([DI, DS], f32, name="acol_t")
            nc.sync.dma_start(acol_t[:], acols[:])
            convb_t = pp.tile([DI, 1], f32, name="convb_t")
            nc.sync.dma_start(convb_t[:], convb[:])
            dtb_t = pp.tile([DI, 1], f32, name="dtb_t")
            nc.sync.dma_start(dtb_t[:], dtb[:])
            dcol_t = pp.tile([DI, 1], f32, name="dcol_t")
            nc.sync.dma_start(dcol_t[:], dcol[:])
            ones1 = pp.tile([65, 128], bf16, name="ones1")
            nc.vector.memset(ones1[:], 1.0)
            # weights
            wc2_t = pp.tile([64, 9 * C], bf16, name="wc2_t")
            nc.sync.dma_start(wc2_t[:], wc2[:])
            wc3_t = pp.tile([64, 2 * C], bf16, name="wc3_t")
            nc.sync.dma_start(wc3_t[:], wc3[:])
            wina_t = pp.tile([128, DI], bf16, name="wina_t")
            nc.sync.dma_start(wina_t[:], wina[:])
            winb_t = pp.tile([128, DI], bf16, name="winb_t")
            nc.sync.dma_start(winb_t[:], winb[:])
            wz_t = pp.tile([64, DI], bf16, name="wz_t")
            nc.sync.dma_start(wz_t[:], wz[:])
            wdt_t = pp.tile([DI, DI], bf16, name="wdt_t")
            nc.sync.dma_start(wdt_t[:], wdt[:])
            wbc_t = pp.tile([DI, 2 * DS], bf16, name="wbc_t")
            nc.sync.dma_start(wbc_t[:], wbc[:])
            wop_t = pp.tile([DI, C], bf16, name="wop_t")
            nc.sync.dma_start(wop_t[:], wop[:])
            wc4_t = pp.tile([64, C], bf16, name="wc4_t")
            nc.sync.dma_start(wc4_t[:], wc4[:])

            # ---- Phase 1: maxpool + conv1 + BN1 -> x2pad ----
            cpool = tc.tile_pool(name="convs", bufs=1)
            cp_ = cpool.__enter__()
            x2pad = cp_.tile([64, 66 * 66], bf16, name="x2pad")
            nc.vector.memset(x2pad[:], 0.0)
            with tc.tile_pool(name="ph1", bufs=1) as p1pool, \
                 tc.tile_pool(name="ph1psum", bufs=1, space="PSUM") as psum1:
                T = p1pool.tile([64, 768], f32, name="T")
                nc.sync.dma_start(
                    T[:].rearrange("p (c hp w) -> p c hp w", c=3, hp=2),
                    ximg.rearrange("c (p hp) w -> p c hp w", hp=2))
                T4 = T[:].rearrange("p (c hp w) -> p c hp w", c=3, hp=2)
                P2 = p1pool.tile([64, 384], f32, name="P2")
                P24 = P2[:].rearrange("p (c hp w) -> p c hp w", c=3, hp=2)
                nc.vector.tensor_tensor(P24, T4[:, :, :, 0:128:2],
                                        T4[:, :, :, 1:128:2], OP.max)
                Pool = p1pool.tile([64, 192], f32, name="Pool")
                P23 = P2[:].rearrange("p (c hp w) -> p c hp w", c=3, hp=2)
                Pl3 = Pool[:].rearrange("p (c w) -> p c w", c=3)
                nc.vector.tensor_tensor(Pl3, P23[:, :, 0, :], P23[:, :, 1, :],
                                        OP.max)
                # stage pooled to DRAM as bf16 (3, 64, 64)
                dpool = dram.tile([3, 64, 64], bf16, name="dpool")
                nc.gpsimd.dma_start(dpool[:].rearrange("c p w -> p c w"), Pl3)
                # X1: partitions (khl, kw, ci); content xpadflat shifted by
                # khl*70 + kw
                X1 = p1pool.tile([63, 4900], bf16, name="X1")
                nc.vector.memset(X1[:], 0.0)
                for khl in range(3):
                    for kw in range(7):
                        p0 = khl * 21 + kw * 3
                        o = (3 - khl) * 70 + (3 - kw)
                        dst = X1[p0:p0 + 3, o:o + 4480]
                        dst3 = dst.rearrange("p (r c) -> p r c", c=70)
                        nc.sync.dma_start(dst3[:, 0:64, 0:64], dpool[:])
                psum_y1 = psum1.tile([C, L], f32, name="psum_y1")
                X1r = X1[:].rearrange("p (r c) -> p r c", c=70)
                y1r = psum_y1[:].rearrange("p (r c) -> p r c", c=64)
                wc1a_t = p1pool.tile([63, C], bf16, name="wc1a_t")
                nc.sync.dma_start(wc1a_t[:], wc1a[:])
                wc1b_t = p1pool.tile([63, C], bf16, name="wc1b_t")
                nc.sync.dma_start(wc1b_t[:], wc1b[:])
                wc1c_t = p1pool.tile([21, C], bf16, name="wc1c_t")
                nc.sync.dma_start(wc1c_t[:], wc1c[:])
                for th in range(NT):
                    r0 = th * 8
                    nc.tensor.matmul(y1r[:, r0:r0 + 8, :],
                                     wc1a_t[:], X1r[:, r0:r0 + 8, 0:64],
                                     start=True, stop=False)
                    nc.tensor.matmul(y1r[:, r0:r0 + 8, :],
                                     wc1b_t[:], X1r[:, 3 + r0:3 + r0 + 8, 0:64],
                                     start=False, stop=False)
                    nc.tensor.matmul(y1r[:, r0:r0 + 8, :],
                                     wc1c_t[0:21, :],
                                     X1r[0:21, 6 + r0:6 + r0 + 8, 0:64],
                                     start=False, stop=True)
                s1, t1 = _bn_block(nc, p1pool, dram, psum_y1[:],
                                   gb[:, 0:1], gb[:, 1:2], L, N_CORES * L, "bn1")
                x2r = x2pad[:].rearrange("p (r c) -> p r c", c=66)
                for th in range(NT):
                    nc.scalar.activation(
                        x2r[:, 1 + th * 8:1 + th * 8 + 8, 1:65],
                        y1r[:, th * 8:th * 8 + 8, :],
                        AF.Relu, bias=t1[:], scale=s1[:])

            # ---- Phase 2: conv2 + BN2 -> xb2 ----
            xb2 = cp_.tile([64, L], bf16, name="xb2")
            with tc.tile_pool(name="ph2", bufs=1) as p2pool, \
                 tc.tile_pool(name="ph2psum", bufs=1, space="PSUM") as psum2:
                psum_y2 = psum2.tile([C, L], f32, name="psum_y2")
                y2r = psum_y2[:].rearrange("p (r c) -> p r c", c=64)
                x2r = x2pad[:].rearrange("p (r c) -> p r c", c=66)
                for th in range(NT):
                    r0 = th * 8
                    first = True
                    for kh in range(3):
                        for kw in range(3):
                            nc.tensor.matmul(
                                y2r[:, r0:r0 + 8, :],
                                wc2_t[:, (kh * 3 + kw) * C:(kh * 3 + kw + 1) * C],
                                x2r[:, kh + r0:kh + r0 + 8, kw:kw + 64],
                                start=first, stop=(kh == 2 and kw == 2))
                            first = False
                s2, t2 = _bn_block(nc, p2pool, dram, psum_y2[:],
                                   gb[:, 2:3], gb[:, 3:4], L, N_CORES * L, "bn2")
                nc.scalar.activation(xb2[:], psum_y2[:], AF.Relu,
                                     bias=t2[:], scale=s2[:])

            # ---- Phase 3: conv3 (dup M=128) + BN3 -> X3s ----
            X3s = pp.tile([128, L + 4], bf16, name="X3s")
            nc.vector.memset(X3s[:], 0.0)
            with tc.tile_pool(name="ph3", bufs=1) as p3pool, \
                 tc.tile_pool(name="ph3psum", bufs=1, space="PSUM") as psum3:
                psum_y3 = psum3.tile([128, L], f32, name="psum_y3")
                for th in range(NT):
                    nc.tensor.matmul(psum_y3[:, th * TS:(th + 1) * TS],
                                     wc3_t[:], xb2[:, th * TS:(th + 1) * TS],
                                     start=True, stop=True)
                s3, t3 = _bn_block(nc, p3pool, dram, psum_y3[0:C, :],
                                   gb[:, 4:5], gb[:, 5:6], L, N_CORES * L, "bn3")
                nc.scalar.activation(X3s[0:64, 3:3 + L], psum_y3[0:64, :],
                                     AF.Relu, bias=t3[:], scale=s3[:])
                nc.scalar.activation(X3s[64:128, 2:2 + L], psum_y3[64:128, :],
                                     AF.Relu, bias=t3[:], scale=s3[:])
            cpool.__exit__(None, None, None)

            # ---- Phase 4: mamba projections ----
            xs_bf = pp.tile([DI, L], bf16, name="xs_bf")
            g_bf = pp.tile([DI, L], bf16, name="g_bf")
            dt_f = pp.tile([DI, L], f32, name="dt_f")
            u_bf = pp.tile([DI, L], bf16, name="u_bf")
            bc_bf = pp.tile([2 * DS, L], bf16, name="bc_bf")
            silu_insts = []
            with tc.tile_pool(name="ph4psum", bufs=2, space="PSUM") as psum4, \
                 tc.tile_pool(name="ph4", bufs=2) as p4pool:
                for th in range(NT):
                    sl = slice(th * TS, (th + 1) * TS)
                    psum_xc = psum4.tile([DI, TS], f32, tag="psum_xc")
                    nc.tensor.matmul(psum_xc[:], wina_t[:],
                                     X3s[:, th * TS:th * TS + TS],
                                     start=True, stop=False)
                    nc.tensor.matmul(psum_xc[:], winb_t[:],
                                     X3s[:, th * TS + 2:th * TS + 2 + TS],
                                     start=False, stop=True)
                    i1 = nc.scalar.activation(xs_bf[:, sl], psum_xc[:],
                                              AF.Silu, bias=convb_t[:])
                    psum_z = psum4.tile([DI, TS], f32, tag="psum_z")
                    nc.tensor.matmul(psum_z[:], wz_t[:],
                                     X3s[0:64, th * TS + 3:th * TS + 3 + TS],
                                     start=True, stop=True)
                    i2 = nc.scalar.activation(g_bf[:, sl], psum_z[:], AF.Silu)
                    silu_insts.append(i1)
                    silu_insts.append(i2)
                for th in range(NT):
                    sl = slice(th * TS, (th + 1) * TS)
                    psum_dt = psum4.tile([DI, TS], f32, tag="psum_dt")
                    nc.tensor.matmul(psum_dt[:], wdt_t[:], xs_bf[:, sl],
                                     start=True, stop=True)
                    et = p4pool.tile([DI, TS], f32, tag="et")
                    e1 = nc.scalar.activation(et[:], psum_dt[:], AF.Exp,
                                              bias=dtb_t[:])
                    nc.scalar.activation(dt_f[:, sl], et[:], AF.Ln, bias=1.0)
                    if th == 0:
                        for si in silu_insts:
                            tile.add_dep_helper(e1.ins, si.ins, sync=False,
                                                reason="act table grouping")
                    psum_bc = psum4.tile([2 * DS, TS], f32, tag="psum_bc")
                    nc.tensor.matmul(psum_bc[:], wbc_t[:], xs_bf[:, sl],
                                     start=True, stop=True)
                    nc.scalar.activation(bc_bf[:, sl], psum_bc[:], AF.Copy)
                    nc.vector.tensor_tensor(u_bf[:, sl], dt_f[:, sl],
                                            xs_bf[:, sl], OP.mult)

            # ---- Phase 5: selective scan, rotated s-assignment ----
            # rotation j: partition p handles state s = (p + j) % 16.
            # B/C replicas come from DRAM staging rows [j, j+128) where
            # staged row r holds B[r % 16].
            b3 = dram.tile([DI + DS - 1, L], bf16, name="b3")
            c3 = dram.tile([DI + DS - 1, L], bf16, name="c3")
            for r0 in range(0, DI + DS - 1, DS):
                n = min(DS, DI + DS - 1 - r0)
                nc.sync.dma_start(b3[r0:r0 + n, :], bc_bf[0:n, :])
                nc.scalar.dma_start(c3[r0:r0 + n, :], bc_bf[DS:DS + n, :])
            y0 = pp.tile([DI, L], bf16, name="y0")
            y1t = pp.tile([DI, L], bf16, name="y1t")
            with tc.tile_pool(name="ph5", bufs=2) as p5pool:
                for s in range(DS):
                    ball = p5pool.tile([DI, L], bf16, tag="ball", bufs=2)
                    nc.sync.dma_start(ball[:], b3[s:s + DI, :])
                    call = p5pool.tile([DI, L], bf16, tag="call", bufs=2)
                    nc.scalar.dma_start(call[:], c3[s:s + DI, :])
                    a_bf = p5pool.tile([DI, L], bf16, tag="a_bf", bufs=2)
                    nc.scalar.activation(a_bf[:], dt_f[:], AF.Exp,
                                         scale=acol_t[:, s:s + 1])
                    b_bf = p5pool.tile([DI, L], bf16, tag="b_bf", bufs=2)
                    h_bf = p5pool.tile([DI, L], bf16, tag="h_bf", bufs=2)
                    p_bf = p5pool.tile([DI, L], bf16, tag="p_bf", bufs=2)
                    yacc = y0 if s % 2 == 0 else y1t
                    if s % 2 == 0:
                        nc.vector.tensor_tensor(b_bf[:], u_bf[:], ball[:], OP.mult)
                        nc.vector.tensor_tensor_scan(h_bf[:], a_bf[:], b_bf[:],
                                                     0.0, OP.mult, OP.add)
                        nc.gpsimd.tensor_tensor(p_bf[:], h_bf[:], call[:], OP.mult)
                        if s < 2:
                            nc.vector.tensor_copy(yacc[:], p_bf[:])
                        else:
                            nc.vector.tensor_tensor(yacc[:], yacc[:], p_bf[:],
                                                    OP.add)
                    else:
                        nc.gpsimd.tensor_tensor(b_bf[:], u_bf[:], ball[:], OP.mult)
                        nc.vector.tensor_tensor_scan(h_bf[:], a_bf[:], b_bf[:],
                                                     0.0, OP.mult, OP.add)
                        nc.vector.tensor_tensor(p_bf[:], h_bf[:], call[:], OP.mult)
                        if s < 2:
                            nc.vector.tensor_copy(yacc[:], p_bf[:])
                        else:
                            nc.gpsimd.tensor_tensor(yacc[:], yacc[:], p_bf[:],
                                                    OP.add)

            # ---- Phase 6: tail ----
            with tc.tile_pool(name="ph6", bufs=1) as p6pool:
                out_f = p6pool.tile([C, L], f32, name="out_f")
                ysum = p6pool.tile([DI, L], bf16, name="ysum")
                nc.vector.tensor_tensor(ysum[:], y0[:], y1t[:], OP.add)
                dx = p6pool.tile([DI, L], bf16, name="dx")
                nc.vector.tensor_scalar_mul(dx[:], xs_bf[:], dcol_t[:])
                y2m = p6pool.tile([DI, L], bf16, name="y2m")
                nc.vector.tensor_tensor(y2m[:], ysum[:], dx[:], OP.add)
                y3m = p6pool.tile([DI, L], bf16, name="y3m")
                nc.vector.tensor_tensor(y3m[:], y2m[:], g_bf[:], OP.mult)
                m_bf = p6pool.tile([C, L], bf16, name="m_bf")
                with tc.tile_pool(name="ph6psum", bufs=2,
                                  space="PSUM") as psum6:
                    for th in range(NT):
                        sl = slice(th * TS, (th + 1) * TS)
                        psum_m = psum6.tile([C, TS], f32, tag="psum_m")
                        nc.tensor.matmul(psum_m[:], wop_t[:], y3m[:, sl],
                                         start=True, stop=True)
                        nc.scalar.activation(m_bf[:, sl], psum_m[:], AF.Copy)
                with tc.tile_pool(name="ph6psum2", bufs=1,
                                  space="PSUM") as psum7:
                    psum_y4 = psum7.tile([C, L], f32, name="psum_y4")
                    for th in range(NT):
                        sl = slice(th * TS, (th + 1) * TS)
                        nc.tensor.matmul(psum_y4[:, sl], wc4_t[:], m_bf[:, sl],
                                         start=True, stop=False)
                        nc.tensor.matmul(psum_y4[:, sl], wc4_t[:],
                                         X3s[0:64, 3 + th * TS:3 + th * TS + TS],
                                         start=False, stop=True)
                    s4, t4 = _bn_block(nc, p6pool, dram, psum_y4[:],
                                       gb[:, 6:7], gb[:, 7:8], L,
                                       N_CORES * L, "bn4")
                    nc.scalar.activation(out_f[:], psum_y4[:], AF.Relu,
                                         bias=t4[:], scale=s4[:])
            nc.sync.dma_start(out_y[:], out_f[:])

    nc.compile()
    return nc


def _prep_weights(I):
    """Host-side weight layout prep (all tiny)."""
    bf = lambda a: np.ascontiguousarray(a, dtype=np.float32).astype(bfnp)
    f = lambda a: np.ascontiguousarray(a, dtype=np.float32)
    w1 = np.asarray(I["w1"], np.float32)      # (64, 3, 7, 7)
    t1 = w1.transpose(2, 3, 1, 0)             # (kh, kw, ci, co)
    # partitions (khl, kw, ci)
    wc1a = t1[0:3].transpose(0, 1, 2, 3).reshape(3, 7, 3, C)
    mk1 = lambda g: np.ascontiguousarray(
        t1[g * 3:g * 3 + 3].reshape(3, 7, 3, C)).reshape(63, C)
    wc1a = mk1(0)
    wc1b = mk1(1)
    wc1c = np.ascontiguousarray(t1[6:7].reshape(1, 7, 3, C)).reshape(21, C)
    w2 = np.asarray(I["w2"], np.float32)
    wc2 = w2.transpose(2, 3, 1, 0).reshape(9, C, C)  # (tap, ci, co)
    wc2 = np.ascontiguousarray(wc2.transpose(1, 0, 2)).reshape(C, 9 * C)
    w3 = np.asarray(I["w3"], np.float32)[:, :, 0, 0]  # (co, ci)
    wc3 = np.concatenate([w3.T, w3.T], axis=1)        # (ci, 2*co)
    in_proj = np.asarray(I["in_proj"], np.float32)    # (256, 64)
    Wxs, Wz = in_proj[:DI], in_proj[DI:]
    cw = np.asarray(I["conv_w"], np.float32)[:, 0, :]  # (di, 4)
    # wina[j*64+c, d] = cw[d, j] * Wxs[d, c], j in {0,1}
    wina = np.empty((128, DI), np.float32)
    winb = np.empty((128, DI), np.float32)
    for j in range(2):
        wina[j * 64:(j + 1) * 64] = (cw[:, j][:, None] * Wxs).T
        winb[j * 64:(j + 1) * 64] = (cw[:, 2 + j][:, None] * Wxs).T
    x_proj = np.asarray(I["x_proj"], np.float32)       # (36, 128)
    dt_pre = np.asarray(I["dt_w"], np.float32) @ x_proj[:R]  # (di, di)
    wdt = dt_pre.T
    wbc = x_proj[R:].T                                  # (128, 32)
    wop = np.asarray(I["out_proj"], np.float32).T       # (128, 64)
    w4 = np.asarray(I["w4"], np.float32)[:, :, 0, 0]
    wc4 = w4.T
    A = -np.exp(np.asarray(I["A_log"], np.float32))     # (di, ds)
    Arot = np.empty((DI, DS), np.float32)
    for j in range(DS):
        Arot[:, j] = A[np.arange(DI), (np.arange(DI) + j) % DS]
    gbcols = np.stack([f(I["g1"]), f(I["b1"]), f(I["g2"]), f(I["b2"]),
                       f(I["g3"]), f(I["b3"]), f(I["g4"]), f(I["b4"])],
                      axis=1)
    return {
        "wc1a": bf(wc1a), "wc1b": bf(wc1b), "wc1c": bf(wc1c),
        "wc2": bf(wc2), "wc3": bf(wc3),
        "wina": bf(wina), "winb": bf(winb), "wz": bf(Wz.T),
        "wdt": bf(wdt), "wbc": bf(wbc), "wop": bf(wop), "wc4": bf(wc4),
        "convb": f(I["conv_b"]).reshape(DI, 1),
        "dtb": f(I["dt_b"]).reshape(DI, 1),
        "acols": f(Arot), "dcol": f(I["D"]).reshape(DI, 1),
        "gbcols": f(gbcols),
    }


def kernel(**inputs):
    if "nc" not in _cache:
        _cache["nc"] = build_program()
    nc = _cache["nc"]
    wmap = _prep_weights(inputs)
    x = np.asarray(inputs["x"], np.float32)  # (8, 3, 128, 128)
    in_maps = []
    for b in range(N_CORES):
        m = dict(wmap)
        m["ximg"] = np.ascontiguousarray(x[b])
        in_maps.append(m)
    import os
    trace = bool(os.environ.get("KERNEL_TRACE"))
    if trace:
        try:
            import trace_shim  # noqa: F401  (dev-only profiling hook)
        except ImportError:
            trace = False
    res = bass_utils.run_bass_kernel_spmd(nc, in_maps,
                                          core_ids=list(range(N_CORES)),
                                          trace=trace)
    _cache["exec_time_ns"] = res.exec_time_ns
    out = np.empty((8, C, H, W), np.float32)
    for b in range(N_CORES):
        out[b] = res.results[b]["out_y"].reshape(C, H, W)
    lnb1 = np.asarray(inputs["lnb1"], np.float32).reshape(1, 1, 1, 1)
    lnb2 = np.asarray(inputs["lnb2"], np.float32).reshape(1, 1, 1, 1)
    p1 = np.broadcast_to(lnb1, (8, 1, H, W)).copy()
    p2 = np.broadcast_to(lnb2, (8, 1, H, W)).copy()
    return out, p1, p2


# revision 11
# speedup vs baseline: 1.1362x; 1.1312x over previous
"""Trainium2 Bass kernel for nn_Block_6012954214590.

Pipeline (per batch element, data-parallel over 8 NeuronCores):
  maxpool2x2 -> conv7x7+BN+ReLU -> conv3x3+BN+ReLU -> conv1x1+BN+ReLU
  -> Mamba block (in_proj [+folded depthwise causal conv], silu, x_proj,
     softplus dt, selective scan over L=4096 via DVE tensor_tensor_scan,
     gate, out_proj) -> residual -> conv1x1+BN+ReLU.
BN uses global batch statistics via tiny cross-core AllReduces (64x2 f32).
p1/p2 outputs are LayerNorm over a singleton channel axis == lnb exactly.
"""
import numpy as np
import ml_dtypes

import concourse.bass as bass
import concourse.bacc as bacc
import concourse.mybir as mybir
import concourse.tile as tile
from concourse import bass_utils

N_CORES = 8
C = 64          # conv channels
H = 64          # post-pool spatial
W = 64
L = H * W       # 4096
DI = 128        # mamba inner dim
DS = 16         # mamba state dim
R = 4           # dt rank
EPS = 1e-5
NT = 8          # N-tiles of 512 over L
TS = 512
f32 = mybir.dt.float32
bf16 = mybir.dt.bfloat16
AF = mybir.ActivationFunctionType
OP = mybir.AluOpType
bfnp = ml_dtypes.bfloat16

_cache = {}


def _bn_block(nc, pool, dram, psum_src, gcol, bcol, n_local, n_global, tag):
    """Compute global-batch BN scale/bias from a psum tensor (64, L).

    Returns (s_col, t_col) f32 (64,1) tiles: out = relu(s*y + t).
    """
    stats6 = pool.tile([C, 8 * 6], f32, name=f"stats6_{tag}")
    src3d = psum_src.rearrange("p (n f) -> p n f", f=TS)
    for i in range(8):
        nc.vector.bn_stats(stats6[:, i * 6:(i + 1) * 6], src3d[:, i, :])
    mv = pool.tile([C, 2], f32, name=f"mv_{tag}")
    nc.vector.bn_aggr(mv[:], stats6[:].rearrange("p (n s) -> p n s", s=6))
    # pack per-core (sum, sumsq)
    packed = pool.tile([C, 2], f32, name=f"packed_{tag}")
    nc.vector.tensor_scalar_mul(packed[:, 0:1], mv[:, 0:1], float(n_local))
    m2 = pool.tile([C, 1], f32, name=f"m2_{tag}")
    nc.vector.tensor_tensor(m2[:], mv[:, 0:1], mv[:, 0:1], OP.mult)
    vp = pool.tile([C, 1], f32, name=f"vp_{tag}")
    nc.vector.tensor_tensor(vp[:], mv[:, 1:2], m2[:], OP.add)
    nc.vector.tensor_scalar_mul(packed[:, 1:2], vp[:], float(n_local))
    # allreduce
    cin = dram.tile([C, 2], f32, name=f"arin_{tag}")
    cout = dram.tile([C, 2], f32, name=f"arout_{tag}")
    nc.sync.dma_start(cin[:], packed[:])
    nc.gpsimd.collective_compute(
        "AllReduce", OP.add, replica_groups=[list(range(N_CORES))],
        ins=[cin.opt()], outs=[cout.opt()],
    )
    glob = pool.tile([C, 2], f32, name=f"glob_{tag}")
    nc.sync.dma_start(glob[:], cout[:])
    # mu, var, rsqrt
    mu = pool.tile([C, 1], f32, name=f"mu_{tag}")
    nc.vector.tensor_scalar_mul(mu[:], glob[:, 0:1], 1.0 / n_global)
    e2 = pool.tile([C, 1], f32, name=f"e2_{tag}")
    nc.vector.tensor_scalar_mul(e2[:], glob[:, 1:2], 1.0 / n_global)
    m2g = pool.tile([C, 1], f32, name=f"m2g_{tag}")
    nc.vector.tensor_tensor(m2g[:], mu[:], mu[:], OP.mult)
    v = pool.tile([C, 1], f32, name=f"v_{tag}")
    nc.vector.tensor_tensor(v[:], e2[:], m2g[:], OP.subtract)
    nc.vector.tensor_scalar_add(v[:], v[:], EPS)
    lnv = pool.tile([C, 1], f32, name=f"lnv_{tag}")
    nc.scalar.activation(lnv[:], v[:], AF.Ln)
    rs = pool.tile([C, 1], f32, name=f"rs_{tag}")
    nc.scalar.activation(rs[:], lnv[:], AF.Exp, scale=-0.5)
    s_col = pool.tile([C, 1], f32, name=f"scol_{tag}")
    nc.vector.tensor_tensor(s_col[:], rs[:], gcol, OP.mult)
    ms = pool.tile([C, 1], f32, name=f"ms_{tag}")
    nc.vector.tensor_tensor(ms[:], mu[:], s_col[:], OP.mult)
    t_col = pool.tile([C, 1], f32, name=f"tcol_{tag}")
    nc.vector.tensor_tensor(t_col[:], bcol, ms[:], OP.subtract)
    return s_col, t_col


def build_program():
    nc = bacc.Bacc("TRN2", target_bir_lowering=False, debug=False,
                   enable_asserts=False, num_devices=N_CORES)

    def din(name, shape, dt):
        return nc.dram_tensor(name, shape, dt, kind="ExternalInput").ap()

    ximg = din("ximg", [3, 128, 128], f32)
    wc1a = din("wc1a", [63, C], bf16)   # rows (khl, kw, ci) for kh 0-2
    wc1b = din("wc1b", [63, C], bf16)   # kh 3-5
    wc1c = din("wc1c", [21, C], bf16)   # kh 6
    wc2 = din("wc2", [64, 9 * C], bf16)  # [ci, (tap co)]
    wc3 = din("wc3", [64, 2 * C], bf16)  # duplicated output channels
    wina = din("wina", [128, DI], bf16)  # (j*64+c, d) taps 0,1 of folded dwconv
    winb = din("winb", [128, DI], bf16)  # taps 2,3
    wz = din("wz", [64, DI], bf16)
    wdt = din("wdt", [DI, DI], bf16)     # dt_w @ x_proj[:4] transposed
    wbc = din("wbc", [DI, 2 * DS], bf16)
    wop = din("wop", [DI, C], bf16)
    wc4 = din("wc4", [64, C], bf16)
    convb = din("convb", [DI, 1], f32)
    dtb = din("dtb", [DI, 1], f32)
    acols = din("acols", [DI, DS], f32)  # A = -exp(A_log)
    dcol = din("dcol", [DI, 1], f32)
    gbcols = din("gbcols", [C, 8], f32)  # g1 b1 g2 b2 g3 b3 g4 b4
    out_y = nc.dram_tensor("out_y", [C, L], f32, kind="ExternalOutput").ap()

    with tile.TileContext(nc) as tc:
        with tc.tile_pool(name="persist", bufs=1) as pp, \
             tc.tile_pool(name="dram", bufs=1, space="DRAM") as dram:
            gb = pp.tile([C, 8], f32, name="gb")
            nc.sync.dma_start(gb[:], gbcols[:])
            acol_t = pp.tile([DI, DS], f32, name="acol_t")
            nc.sync.dma_start(acol_t[:], acols[:])
            convb_t = pp.tile([DI, 1], f32, name="convb_t")
            nc.sync.dma_start(convb_t[:], convb[:])
            dtb_t = pp.tile([DI, 1], f32, name="dtb_t")
            nc.sync.dma_start(dtb_t[:], dtb[:])
            dcol_t = pp.tile([DI, 1], f32, name="dcol_t")
            nc.sync.dma_start(dcol_t[:], dcol[:])
            ones1 = pp.tile([65, 128], bf16, name="ones1")
            nc.vector.memset(ones1[:], 1.0)
            warm = pp.tile([C, 2], f32, name="warm")
            nc.vector.memset(warm[:], 0.0)
            win_d = dram.tile([C, 2], f32, name="warm_in")
            wout_d = dram.tile([C, 2], f32, name="warm_out")
            nc.sync.dma_start(win_d[:], warm[:])
            nc.gpsimd.collective_compute(
                "AllReduce", OP.add, replica_groups=[list(range(N_CORES))],
                ins=[win_d.opt()], outs=[wout_d.opt()],
            )
            # weights
            wc2_t = pp.tile([64, 9 * C], bf16, name="wc2_t")
            nc.sync.dma_start(wc2_t[:], wc2[:])
            wc3_t = pp.tile([64, 2 * C], bf16, name="wc3_t")
            nc.sync.dma_start(wc3_t[:], wc3[:])
            wina_t = pp.tile([128, DI], bf16, name="wina_t")
            nc.sync.dma_start(wina_t[:], wina[:])
            winb_t = pp.tile([128, DI], bf16, name="winb_t")
            nc.sync.dma_start(winb_t[:], winb[:])
            wz_t = pp.tile([64, DI], bf16, name="wz_t")
            nc.sync.dma_start(wz_t[:], wz[:])
            wdt_t = pp.tile([DI, DI], bf16, name="wdt_t")
            nc.sync.dma_start(wdt_t[:], wdt[:])
            wbc_t = pp.tile([DI, 2 * DS], bf16, name="wbc_t")
            nc.sync.dma_start(wbc_t[:], wbc[:])
            wop_t = pp.tile([DI, C], bf16, name="wop_t")
            nc.sync.dma_start(wop_t[:], wop[:])
            wc4_t = pp.tile([64, C], bf16, name="wc4_t")
            nc.sync.dma_start(wc4_t[:], wc4[:])

            # ---- Phase 1: maxpool + conv1 + BN1 -> x2pad ----
            cpool = tc.tile_pool(name="convs", bufs=1)
            cp_ = cpool.__enter__()
            x2pad = cp_.tile([64, 66 * 66], bf16, name="x2pad")
            nc.vector.memset(x2pad[:], 0.0)
            with tc.tile_pool(name="ph1", bufs=1) as p1pool, \
                 tc.tile_pool(name="ph1psum", bufs=1, space="PSUM") as psum1:
                T = p1pool.tile([64, 768], f32, name="T")
                nc.sync.dma_start(
                    T[:].rearrange("p (c hp w) -> p c hp w", c=3, hp=2),
                    ximg.rearrange("c (p hp) w -> p c hp w", hp=2))
                T4 = T[:].rearrange("p (c hp w) -> p c hp w", c=3, hp=2)
                P2 = p1pool.tile([64, 384], f32, name="P2")
                P24 = P2[:].rearrange("p (c hp w) -> p c hp w", c=3, hp=2)
                nc.vector.tensor_tensor(P24, T4[:, :, :, 0:128:2],
                                        T4[:, :, :, 1:128:2], OP.max)
                Pool = p1pool.tile([64, 192], f32, name="Pool")
                P23 = P2[:].rearrange("p (c hp w) -> p c hp w", c=3, hp=2)
                Pl3 = Pool[:].rearrange("p (c w) -> p c w", c=3)
                nc.vector.tensor_tensor(Pl3, P23[:, :, 0, :], P23[:, :, 1, :],
                                        OP.max)
                # stage pooled to DRAM as bf16 (3, 64, 64)
                dpool = dram.tile([3, 64, 64], bf16, name="dpool")
                nc.gpsimd.dma_start(dpool[:].rearrange("c p w -> p c w"), Pl3)
                # X1: partitions (khl, kw, ci); content xpadflat shifted by
                # khl*70 + kw
                X1 = p1pool.tile([63, 4900], bf16, name="X1")
                nc.vector.memset(X1[:], 0.0)
                for khl in range(3):
                    for kw in range(7):
                        p0 = khl * 21 + kw * 3
                        o = (3 - khl) * 70 + (3 - kw)
                        dst = X1[p0:p0 + 3, o:o + 4480]
                        dst3 = dst.rearrange("p (r c) -> p r c", c=70)
                        nc.sync.dma_start(dst3[:, 0:64, 0:64], dpool[:])
                psum_y1 = psum1.tile([C, L], f32, name="psum_y1")
                X1r = X1[:].rearrange("p (r c) -> p r c", c=70)
                y1r = psum_y1[:].rearrange("p (r c) -> p r c", c=64)
                wc1a_t = p1pool.tile([63, C], bf16, name="wc1a_t")
                nc.sync.dma_start(wc1a_t[:], wc1a[:])
                wc1b_t = p1pool.tile([63, C], bf16, name="wc1b_t")
                nc.sync.dma_start(wc1b_t[:], wc1b[:])
                wc1c_t = p1pool.tile([21, C], bf16, name="wc1c_t")
                nc.sync.dma_start(wc1c_t[:], wc1c[:])
                for th in range(NT):
                    r0 = th * 8
                    nc.tensor.matmul(y1r[:, r0:r0 + 8, :],
                                     wc1a_t[:], X1r[:, r0:r0 + 8, 0:64],
                                     start=True, stop=False)
                    nc.tensor.matmul(y1r[:, r0:r0 + 8, :],
                                     wc1b_t[:], X1r[:, 3 + r0:3 + r0 + 8, 0:64],
                                     start=False, stop=False)
                    nc.tensor.matmul(y1r[:, r0:r0 + 8, :],
                                     wc1c_t[0:21, :],
                                     X1r[0:21, 6 + r0:6 + r0 + 8, 0:64],
                                     start=False, stop=True)
                s1, t1 = _bn_block(nc, p1pool, dram, psum_y1[:],
                                   gb[:, 0:1], gb[:, 1:2], L, N_CORES * L, "bn1")
                x2r = x2pad[:].rearrange("p (r c) -> p r c", c=66)
                for th in range(NT):
                    nc.scalar.activation(
                        x2r[:, 1 + th * 8:1 + th * 8 + 8, 1:65],
                        y1r[:, th * 8:th * 8 + 8, :],
                        AF.Relu, bias=t1[:], scale=s1[:])

            # ---- Phase 2: conv2 + BN2 -> xb2 ----
            xb2 = cp_.tile([64, L], bf16, name="xb2")
            with tc.tile_pool(name="ph2", bufs=1) as p2pool, \
                 tc.tile_pool(name="ph2psum", bufs=1, space="PSUM") as psum2:
                psum_y2 = psum2.tile([C, L], f32, name="psum_y2")
                y2r = psum_y2[:].rearrange("p (r c) -> p r c", c=64)
                x2r = x2pad[:].rearrange("p (r c) -> p r c", c=66)
                for th in range(NT):
                    r0 = th * 8
                    first = True
                    for kh in range(3):
                        for kw in range(3):
                            nc.tensor.matmul(
                                y2r[:, r0:r0 + 8, :],
                                wc2_t[:, (kh * 3 + kw) * C:(kh * 3 + kw + 1) * C],
                                x2r[:, kh + r0:kh + r0 + 8, kw:kw + 64],
                                start=first, stop=(kh == 2 and kw == 2))
                            first = False
                s2, t2 = _bn_block(nc, p2pool, dram, psum_y2[:],
                                   gb[:, 2:3], gb[:, 3:4], L, N_CORES * L, "bn2")
                nc.scalar.activation(xb2[:], psum_y2[:], AF.Relu,
                                     bias=t2[:], scale=s2[:])

            # ---- Phase 3: conv3 (dup M=128) + BN3 -> X3s ----
            X3s = pp.tile([128, L + 4], bf16, name="X3s")
            nc.vector.memset(X3s[:], 0.0)
            with tc.tile_pool(name="ph3", bufs=1) as p3pool, \
                 tc.tile_pool(name="ph3psum", bufs=1, space="PSUM") as psum3:
                psum_y3 = psum3.tile([128, L], f32, name="psum_y3")
                for th in range(NT):
                    nc.tensor.matmul(psum_y3[:, th * TS:(th + 1) * TS],
                                     wc3_t[:], xb2[:, th * TS:(th + 1) * TS],
                                     start=True, stop=True)
                s3, t3 = _bn_block(nc, p3pool, dram, psum_y3[0:C, :],
                                   gb[:, 4:5], gb[:, 5:6], L, N_CORES * L, "bn3")
                nc.scalar.activation(X3s[0:64, 3:3 + L], psum_y3[0:64, :],
                                     AF.Relu, bias=t3[:], scale=s3[:])
                nc.scalar.activation(X3s[64:128, 2:2 + L], psum_y3[64:128, :],
                                     AF.Relu, bias=t3[:], scale=s3[:])
            cpool.__exit__(None, None, None)

            # ---- Phase 4: mamba projections ----
            xs_bf = pp.tile([DI, L], bf16, name="xs_bf")
            g_bf = pp.tile([DI, L], bf16, name="g_bf")
            dt_f = pp.tile([DI, L], f32, name="dt_f")
            u_bf = pp.tile([DI, L], bf16, name="u_bf")
            bc_bf = pp.tile([2 * DS, L], bf16, name="bc_bf")
            silu_insts = []
            with tc.tile_pool(name="ph4psum", bufs=2, space="PSUM") as psum4, \
                 tc.tile_pool(name="ph4", bufs=2) as p4pool:
                for th in range(NT):
                    sl = slice(th * TS, (th + 1) * TS)
                    psum_xc = psum4.tile([DI, TS], f32, tag="psum_xc")
                    nc.tensor.matmul(psum_xc[:], wina_t[:],
                                     X3s[:, th * TS:th * TS + TS],
                                     start=True, stop=False)
                    nc.tensor.matmul(psum_xc[:], winb_t[:],
                                     X3s[:, th * TS + 2:th * TS + 2 + TS],
                                     start=False, stop=True)
                    i1 = nc.scalar.activation(xs_bf[:, sl], psum_xc[:],
                                              AF.Silu, bias=convb_t[:])
                    psum_z = psum4.tile([DI, TS], f32, tag="psum_z")
                    nc.tensor.matmul(psum_z[:], wz_t[:],
                                     X3s[0:64, th * TS + 3:th * TS + 3 + TS],
                                     start=True, stop=True)
                    i2 = nc.scalar.activation(g_bf[:, sl], psum_z[:], AF.Silu)
                    silu_insts.append(i1)
                    silu_insts.append(i2)
                for th in range(NT):
                    sl = slice(th * TS, (th + 1) * TS)
                    psum_dt = psum4.tile([DI, TS], f32, tag="psum_dt")
                    nc.tensor.matmul(psum_dt[:], wdt_t[:], xs_bf[:, sl],
                                     start=True, stop=True)
                    et = p4pool.tile([DI, TS], f32, tag="et")
                    e1 = nc.scalar.activation(et[:], psum_dt[:], AF.Exp,
                                              bias=dtb_t[:])
                    nc.scalar.activation(dt_f[:, sl], et[:], AF.Ln, bias=1.0)
                    if th == 0:
                        for si in silu_insts:
                            tile.add_dep_helper(e1.ins, si.ins, sync=False,
                                                reason="act table grouping")
                    psum_bc = psum4.tile([2 * DS, TS], f32, tag="psum_bc")
                    nc.tensor.matmul(psum_bc[:], wbc_t[:], xs_bf[:, sl],
                                     start=True, stop=True)
                    nc.vector.tensor_copy(bc_bf[:, sl], psum_bc[:])
                    nc.vector.tensor_tensor(u_bf[:, sl], dt_f[:, sl],
                                            xs_bf[:, sl], OP.mult)

            # ---- Phase 5: selective scan, rotated s-assignment ----
            # rotation j: partition p handles state s = (p + j) % 16.
            # B/C replicas come from DRAM staging rows [j, j+128) where
            # staged row r holds B[r % 16].
            b3 = dram.tile([DI + DS - 1, L], bf16, name="b3")
            c3 = dram.tile([DI + DS - 1, L], bf16, name="c3")
            for r0 in range(0, DI + DS - 1, DS):
                n = min(DS, DI + DS - 1 - r0)
                nc.sync.dma_start(b3[r0:r0 + n, :], bc_bf[0:n, :])
                nc.scalar.dma_start(c3[r0:r0 + n, :], bc_bf[DS:DS + n, :])
            y0 = pp.tile([DI, L], bf16, name="y0")
            y1t = pp.tile([DI, L], bf16, name="y1t")
            with tc.tile_pool(name="ph5", bufs=2) as p5pool:
                for s in range(DS):
                    ball = p5pool.tile([DI, L], bf16, tag="ball", bufs=2)
                    nc.sync.dma_start(ball[:], b3[s:s + DI, :])
                    call = p5pool.tile([DI, L], bf16, tag="call", bufs=2)
                    nc.scalar.dma_start(call[:], c3[s:s + DI, :])
                    a_bf = p5pool.tile([DI, L], bf16, tag="a_bf", bufs=2)
                    nc.scalar.activation(a_bf[:], dt_f[:], AF.Exp,
                                         scale=acol_t[:, s:s + 1])
                    b_bf = p5pool.tile([DI, L], bf16, tag="b_bf", bufs=4)
                    h_bf = p5pool.tile([DI, L], bf16, tag="h_bf", bufs=2)
                    p_bf = p5pool.tile([DI, L], bf16, tag="p_bf", bufs=2)
                    nc.gpsimd.tensor_tensor(b_bf[:], u_bf[:], ball[:], OP.mult)
                    nc.vector.tensor_tensor_scan(h_bf[:], a_bf[:], b_bf[:],
                                                 0.0, OP.mult, OP.add)
                    nc.vector.tensor_tensor(p_bf[:], h_bf[:], call[:], OP.mult)
                    yacc = y0 if s % 2 == 0 else y1t
                    if s < 2:
                        nc.vector.tensor_copy(yacc[:], p_bf[:])
                    else:
                        nc.vector.tensor_tensor(yacc[:], yacc[:], p_bf[:],
                                                OP.add)

            # ---- Phase 6: tail ----
            with tc.tile_pool(name="ph6", bufs=1) as p6pool:
                out_f = p6pool.tile([C, L], f32, name="out_f")
                ysum = p6pool.tile([DI, L], bf16, name="ysum")
                nc.vector.tensor_tensor(ysum[:], y0[:], y1t[:], OP.add)
                dx = p6pool.tile([DI, L], bf16, name="dx")
                nc.vector.tensor_scalar_mul(dx[:], xs_bf[:], dcol_t[:])
                y2m = p6pool.tile([DI, L], bf16, name="y2m")
                nc.vector.tensor_tensor(y2m[:], ysum[:], dx[:], OP.add)
                y3m = p6pool.tile([DI, L], bf16, name="y3m")
                nc.vector.tensor_tensor(y3m[:], y2m[:], g_bf[:], OP.mult)
                m_bf = p6pool.tile([C, L], bf16, name="m_bf")
                with tc.tile_pool(name="ph6psum", bufs=2,
                                  space="PSUM") as psum6:
                    for th in range(NT):
                        sl = slice(th * TS, (th + 1) * TS)
                        psum_m = psum6.tile([C, TS], f32, tag="psum_m")
                        nc.tensor.matmul(psum_m[:], wop_t[:], y3m[:, sl],
                                         start=True, stop=True)
                        nc.vector.tensor_copy(m_bf[:, sl], psum_m[:])
                with tc.tile_pool(name="ph6psum2", bufs=1,
                                  space="PSUM") as psum7:
                    psum_y4 = psum7.tile([C, L], f32, name="psum_y4")
                    for th in range(NT):
                        sl = slice(th * TS, (th + 1) * TS)
                        nc.tensor.matmul(psum_y4[:, sl], wc4_t[:], m_bf[:, sl],
                                         start=True, stop=False)
                        nc.tensor.matmul(psum_y4[:, sl], wc4_t[:],
                                         X3s[0:64, 3 + th * TS:3 + th * TS + TS],
                                         start=False, stop=True)
                    s4, t4 = _bn_block(nc, p6pool, dram, psum_y4[:],
                                       gb[:, 6:7], gb[:, 7:8], L,
                                       N_CORES * L, "bn4")
                    nc.scalar.activation(out_f[:], psum_y4[:], AF.Relu,
                                         bias=t4[:], scale=s4[:])
            nc.sync.dma_start(out_y[:], out_f[:])

    nc.compile()
    return nc


def _prep_weights(I):
    """Host-side weight layout prep (all tiny)."""
    bf = lambda a: np.ascontiguousarray(a, dtype=np.float32).astype(bfnp)
    f = lambda a: np.ascontiguousarray(a, dtype=np.float32)
    w1 = np.asarray(I["w1"], np.float32)      # (64, 3, 7, 7)
    t1 = w1.transpose(2, 3, 1, 0)             # (kh, kw, ci, co)
    # partitions (khl, kw, ci)
    wc1a = t1[0:3].transpose(0, 1, 2, 3).reshape(3, 7, 3, C)
    mk1 = lambda g: np.ascontiguousarray(
        t1[g * 3:g * 3 + 3].reshape(3, 7, 3, C)).reshape(63, C)
    wc1a = mk1(0)
    wc1b = mk1(1)
    wc1c = np.ascontiguousarray(t1[6:7].reshape(1, 7, 3, C)).reshape(21, C)
    w2 = np.asarray(I["w2"], np.float32)
    wc2 = w2.transpose(2, 3, 1, 0).reshape(9, C, C)  # (tap, ci, co)
    wc2 = np.ascontiguousarray(wc2.transpose(1, 0, 2)).reshape(C, 9 * C)
    w3 = np.asarray(I["w3"], np.float32)[:, :, 0, 0]  # (co, ci)
    wc3 = np.concatenate([w3.T, w3.T], axis=1)        # (ci, 2*co)
    in_proj = np.asarray(I["in_proj"], np.float32)    # (256, 64)
    Wxs, Wz = in_proj[:DI], in_proj[DI:]
    cw = np.asarray(I["conv_w"], np.float32)[:, 0, :]  # (di, 4)
    # wina[j*64+c, d] = cw[d, j] * Wxs[d, c], j in {0,1}
    wina = np.empty((128, DI), np.float32)
    winb = np.empty((128, DI), np.float32)
    for j in range(2):
        wina[j * 64:(j + 1) * 64] = (cw[:, j][:, None] * Wxs).T
        winb[j * 64:(j + 1) * 64] = (cw[:, 2 + j][:, None] * Wxs).T
    x_proj = np.asarray(I["x_proj"], np.float32)       # (36, 128)
    dt_pre = np.asarray(I["dt_w"], np.float32) @ x_proj[:R]  # (di, di)
    wdt = dt_pre.T
    wbc = x_proj[R:].T                                  # (128, 32)
    wop = np.asarray(I["out_proj"], np.float32).T       # (128, 64)
    w4 = np.asarray(I["w4"], np.float32)[:, :, 0, 0]
    wc4 = w4.T
    A = -np.exp(np.asarray(I["A_log"], np.float32))     # (di, ds)
    Arot = np.empty((DI, DS), np.float32)
    for j in range(DS):
        Arot[:, j] = A[np.arange(DI), (np.arange(DI) + j) % DS]
    gbcols = np.stack([f(I["g1"]), f(I["b1"]), f(I["g2"]), f(I["b2"]),
                       f(I["g3"]), f(I["b3"]), f(I["g4"]), f(I["b4"])],
                      axis=1)
    return {
        "wc1a": bf(wc1a), "wc1b": bf(wc1b), "wc1c": bf(wc1c),
        "wc2": bf(wc2), "wc3": bf(wc3),
        "wina": bf(wina), "winb": bf(winb), "wz": bf(Wz.T),
        "wdt": bf(wdt), "wbc": bf(wbc), "wop": bf(wop), "wc4": bf(wc4),
        "convb": f(I["conv_b"]).reshape(DI, 1),
        "dtb": f(I["dt_b"]).reshape(DI, 1),
        "acols": f(Arot), "dcol": f(I["D"]).reshape(DI, 1),
        "gbcols": f(gbcols),
    }


def kernel(**inputs):
    if "nc" not in _cache:
        _cache["nc"] = build_program()
    nc = _cache["nc"]
    wmap = _prep_weights(inputs)
    x = np.asarray(inputs["x"], np.float32)  # (8, 3, 128, 128)
    in_maps = []
    for b in range(N_CORES):
        m = dict(wmap)
        m["ximg"] = np.ascontiguousarray(x[b])
        in_maps.append(m)
    import os
    trace = bool(os.environ.get("KERNEL_TRACE"))
    if trace:
        try:
            import trace_shim  # noqa: F401  (dev-only profiling hook)
        except ImportError:
            trace = False
    res = bass_utils.run_bass_kernel_spmd(nc, in_maps,
                                          core_ids=list(range(N_CORES)),
                                          trace=trace)
    _cache["exec_time_ns"] = res.exec_time_ns
    out = np.empty((8, C, H, W), np.float32)
    for b in range(N_CORES):
        out[b] = res.results[b]["out_y"].reshape(C, H, W)
    lnb1 = np.asarray(inputs["lnb1"], np.float32).reshape(1, 1, 1, 1)
    lnb2 = np.asarray(inputs["lnb2"], np.float32).reshape(1, 1, 1, 1)
    p1 = np.broadcast_to(lnb1, (8, 1, H, W)).copy()
    p2 = np.broadcast_to(lnb2, (8, 1, H, W)).copy()
    return out, p1, p2


# revision 13
# speedup vs baseline: 1.2775x; 1.1244x over previous
"""Trainium2 Bass kernel for nn_Block_6012954214590.

Pipeline (per batch element, data-parallel over 8 NeuronCores):
  maxpool2x2 -> conv7x7+BN+ReLU -> conv3x3+BN+ReLU -> conv1x1+BN+ReLU
  -> Mamba block (in_proj [+folded depthwise causal conv], silu, x_proj,
     softplus dt, selective scan over L=4096 via DVE tensor_tensor_scan,
     gate, out_proj) -> residual -> conv1x1+BN+ReLU.
BN uses global batch statistics via tiny cross-core AllReduces (64x2 f32).
p1/p2 outputs are LayerNorm over a singleton channel axis == lnb exactly.
"""
import numpy as np
import ml_dtypes

import concourse.bass as bass
import concourse.bacc as bacc
import concourse.mybir as mybir
import concourse.tile as tile
from concourse import bass_utils

N_CORES = 8
C = 64          # conv channels
H = 64          # post-pool spatial
W = 64
L = H * W       # 4096
DI = 128        # mamba inner dim
DS = 16         # mamba state dim
R = 4           # dt rank
EPS = 1e-5
NT = 8          # N-tiles of 512 over L
TS = 512
f32 = mybir.dt.float32
bf16 = mybir.dt.bfloat16
AF = mybir.ActivationFunctionType
OP = mybir.AluOpType
bfnp = ml_dtypes.bfloat16

_cache = {}


def _bn_block(nc, pool, dram, psum_src, gcol, bcol, n_local, n_global, tag):
    """Compute global-batch BN scale/bias from a psum tensor (64, L).

    Returns (s_col, t_col) f32 (64,1) tiles: out = relu(s*y + t).
    """
    stats6 = pool.tile([C, 8 * 6], f32, name=f"stats6_{tag}")
    src3d = psum_src.rearrange("p (n f) -> p n f", f=TS)
    for i in range(8):
        nc.vector.bn_stats(stats6[:, i * 6:(i + 1) * 6], src3d[:, i, :])
    mv = pool.tile([C, 2], f32, name=f"mv_{tag}")
    nc.vector.bn_aggr(mv[:], stats6[:].rearrange("p (n s) -> p n s", s=6))
    # pack per-core (sum, sumsq)
    packed = pool.tile([C, 2], f32, name=f"packed_{tag}")
    nc.vector.tensor_scalar_mul(packed[:, 0:1], mv[:, 0:1], float(n_local))
    m2 = pool.tile([C, 1], f32, name=f"m2_{tag}")
    nc.vector.tensor_tensor(m2[:], mv[:, 0:1], mv[:, 0:1], OP.mult)
    vp = pool.tile([C, 1], f32, name=f"vp_{tag}")
    nc.vector.tensor_tensor(vp[:], mv[:, 1:2], m2[:], OP.add)
    nc.vector.tensor_scalar_mul(packed[:, 1:2], vp[:], float(n_local))
    # allreduce
    cin = dram.tile([C, 2], f32, name=f"arin_{tag}")
    cout = dram.tile([C, 2], f32, name=f"arout_{tag}")
    nc.sync.dma_start(cin[:], packed[:])
    nc.gpsimd.collective_compute(
        "AllReduce", OP.add, replica_groups=[list(range(N_CORES))],
        ins=[cin.opt()], outs=[cout.opt()],
    )
    glob = pool.tile([C, 2], f32, name=f"glob_{tag}")
    nc.sync.dma_start(glob[:], cout[:])
    # mu, var, rsqrt
    mu = pool.tile([C, 1], f32, name=f"mu_{tag}")
    nc.vector.tensor_scalar_mul(mu[:], glob[:, 0:1], 1.0 / n_global)
    e2 = pool.tile([C, 1], f32, name=f"e2_{tag}")
    nc.vector.tensor_scalar_mul(e2[:], glob[:, 1:2], 1.0 / n_global)
    m2g = pool.tile([C, 1], f32, name=f"m2g_{tag}")
    nc.vector.tensor_tensor(m2g[:], mu[:], mu[:], OP.mult)
    v = pool.tile([C, 1], f32, name=f"v_{tag}")
    nc.vector.tensor_tensor(v[:], e2[:], m2g[:], OP.subtract)
    nc.vector.tensor_scalar_add(v[:], v[:], EPS)
    # rsqrt via bit-hack seed + 3 Newton iterations (no ACT tables needed)
    i32 = mybir.dt.int32
    magic = pool.tile([C, 1], i32, name=f"magic_{tag}")
    nc.vector.memset(magic[:], 0x5f3759df)
    half = pool.tile([C, 1], i32, name=f"half_{tag}")
    nc.vector.tensor_scalar(half[:], v[:].bitcast(i32), 1, None,
                            OP.logical_shift_right)
    rs = pool.tile([C, 1], f32, name=f"rs_{tag}")
    nc.vector.tensor_tensor(rs[:].bitcast(i32), magic[:], half[:], OP.subtract)
    tmp = pool.tile([C, 1], f32, name=f"nt_{tag}")
    for _ in range(3):
        nc.vector.tensor_tensor(tmp[:], rs[:], rs[:], OP.mult)
        nc.vector.tensor_tensor(tmp[:], tmp[:], v[:], OP.mult)
        nc.vector.tensor_scalar(tmp[:], tmp[:], -0.5, 1.5, OP.mult, OP.add)
        nc.vector.tensor_tensor(rs[:], rs[:], tmp[:], OP.mult)
    s_col = pool.tile([C, 1], f32, name=f"scol_{tag}")
    nc.vector.tensor_tensor(s_col[:], rs[:], gcol, OP.mult)
    ms = pool.tile([C, 1], f32, name=f"ms_{tag}")
    nc.vector.tensor_tensor(ms[:], mu[:], s_col[:], OP.mult)
    t_col = pool.tile([C, 1], f32, name=f"tcol_{tag}")
    nc.vector.tensor_tensor(t_col[:], bcol, ms[:], OP.subtract)
    return s_col, t_col


def build_program():
    nc = bacc.Bacc("TRN2", target_bir_lowering=False, debug=False,
                   enable_asserts=False, num_devices=N_CORES)

    def din(name, shape, dt):
        return nc.dram_tensor(name, shape, dt, kind="ExternalInput").ap()

    ximg = din("ximg", [3, 128, 128], f32)
    wc1a = din("wc1a", [63, C], bf16)   # rows (khl, kw, ci) for kh 0-2
    wc1b = din("wc1b", [63, C], bf16)   # kh 3-5
    wc1c = din("wc1c", [21, C], bf16)   # kh 6
    wc2 = din("wc2", [64, 9 * C], bf16)  # [ci, (tap co)]
    wc3 = din("wc3", [64, 2 * C], bf16)  # duplicated output channels
    wina = din("wina", [128, DI], bf16)  # (j*64+c, d) taps 0,1 of folded dwconv
    winb = din("winb", [128, DI], bf16)  # taps 2,3
    wz = din("wz", [64, DI], bf16)
    wdt = din("wdt", [DI, DI], bf16)     # dt_w @ x_proj[:4] transposed
    wbc = din("wbc", [DI, 2 * DS], bf16)
    wop = din("wop", [DI, C], bf16)
    wc4 = din("wc4", [64, C], bf16)
    convb = din("convb", [DI, 1], f32)
    dtb = din("dtb", [DI, 1], f32)
    acols = din("acols", [DI, DS], f32)  # A = -exp(A_log)
    dcol = din("dcol", [DI, 1], f32)
    gbcols = din("gbcols", [C, 8], f32)  # g1 b1 g2 b2 g3 b3 g4 b4
    out_y = nc.dram_tensor("out_y", [C, L], f32, kind="ExternalOutput").ap()

    with tile.TileContext(nc) as tc:
        with tc.tile_pool(name="persist", bufs=1) as pp, \
             tc.tile_pool(name="dram", bufs=1, space="DRAM") as dram:
            gb = pp.tile([C, 8], f32, name="gb")
            nc.sync.dma_start(gb[:], gbcols[:])
            acol_t = pp.tile([DI, DS], f32, name="acol_t")
            nc.sync.dma_start(acol_t[:], acols[:])
            convb_t = pp.tile([DI, 1], f32, name="convb_t")
            nc.sync.dma_start(convb_t[:], convb[:])
            dtb_t = pp.tile([DI, 1], f32, name="dtb_t")
            nc.sync.dma_start(dtb_t[:], dtb[:])
            dcol_t = pp.tile([DI, 1], f32, name="dcol_t")
            nc.sync.dma_start(dcol_t[:], dcol[:])
            ones1 = pp.tile([65, 128], bf16, name="ones1")
            nc.vector.memset(ones1[:], 1.0)
            warm = pp.tile([C, 2], f32, name="warm")
            nc.vector.memset(warm[:], 0.0)
            win_d = dram.tile([C, 2], f32, name="warm_in")
            wout_d = dram.tile([C, 2], f32, name="warm_out")
            nc.sync.dma_start(win_d[:], warm[:])
            nc.gpsimd.collective_compute(
                "AllReduce", OP.add, replica_groups=[list(range(N_CORES))],
                ins=[win_d.opt()], outs=[wout_d.opt()],
            )
            # weights
            wc2_t = pp.tile([64, 9 * C], bf16, name="wc2_t")
            nc.sync.dma_start(wc2_t[:], wc2[:])
            wc3_t = pp.tile([64, 2 * C], bf16, name="wc3_t")
            nc.sync.dma_start(wc3_t[:], wc3[:])
            wina_t = pp.tile([128, DI], bf16, name="wina_t")
            nc.sync.dma_start(wina_t[:], wina[:])
            winb_t = pp.tile([128, DI], bf16, name="winb_t")
            nc.sync.dma_start(winb_t[:], winb[:])
            wz_t = pp.tile([64, DI], bf16, name="wz_t")
            nc.sync.dma_start(wz_t[:], wz[:])
            wdt_t = pp.tile([DI, DI], bf16, name="wdt_t")
            nc.sync.dma_start(wdt_t[:], wdt[:])
            wbc_t = pp.tile([DI, 2 * DS], bf16, name="wbc_t")
            nc.sync.dma_start(wbc_t[:], wbc[:])
            wop_t = pp.tile([DI, C], bf16, name="wop_t")
            nc.sync.dma_start(wop_t[:], wop[:])
            wc4_t = pp.tile([64, C], bf16, name="wc4_t")
            nc.sync.dma_start(wc4_t[:], wc4[:])

            # ---- Phase 1: maxpool + conv1 + BN1 -> x2pad ----
            cpool = tc.tile_pool(name="convs", bufs=1)
            cp_ = cpool.__enter__()
            x2pad = cp_.tile([64, 66 * 66], bf16, name="x2pad")
            nc.vector.memset(x2pad[:], 0.0)
            with tc.tile_pool(name="ph1", bufs=1) as p1pool, \
                 tc.tile_pool(name="ph1psum", bufs=1, space="PSUM") as psum1:
                T = p1pool.tile([64, 768], f32, name="T")
                nc.sync.dma_start(
                    T[:].rearrange("p (c hp w) -> p c hp w", c=3, hp=2),
                    ximg.rearrange("c (p hp) w -> p c hp w", hp=2))
                T4 = T[:].rearrange("p (c hp w) -> p c hp w", c=3, hp=2)
                P2 = p1pool.tile([64, 384], f32, name="P2")
                P24 = P2[:].rearrange("p (c hp w) -> p c hp w", c=3, hp=2)
                nc.vector.tensor_tensor(P24, T4[:, :, :, 0:128:2],
                                        T4[:, :, :, 1:128:2], OP.max)
                Pool = p1pool.tile([64, 192], f32, name="Pool")
                P23 = P2[:].rearrange("p (c hp w) -> p c hp w", c=3, hp=2)
                Pl3 = Pool[:].rearrange("p (c w) -> p c w", c=3)
                nc.vector.tensor_tensor(Pl3, P23[:, :, 0, :], P23[:, :, 1, :],
                                        OP.max)
                # stage pooled to DRAM as bf16 (3, 64, 64)
                dpool = dram.tile([3, 64, 64], bf16, name="dpool")
                nc.gpsimd.dma_start(dpool[:].rearrange("c p w -> p c w"), Pl3)
                # X1: partitions (khl, kw, ci); content xpadflat shifted by
                # khl*70 + kw
                X1 = p1pool.tile([63, 4900], bf16, name="X1")
                nc.vector.memset(X1[:], 0.0)
                qi = 0
                for khl in range(3):
                    for kw in range(7):
                        p0 = khl * 21 + kw * 3
                        o = (3 - khl) * 70 + (3 - kw)
                        dst = X1[p0:p0 + 3, o:o + 4480]
                        dst3 = dst.rearrange("p (r c) -> p r c", c=70)
                        eng = (nc.sync, nc.scalar, nc.gpsimd)[qi % 3]
                        eng.dma_start(dst3[:, 0:64, 0:64], dpool[:])
                        qi += 1
                psum_y1 = psum1.tile([C, L], f32, name="psum_y1")
                X1r = X1[:].rearrange("p (r c) -> p r c", c=70)
                y1r = psum_y1[:].rearrange("p (r c) -> p r c", c=64)
                wc1a_t = p1pool.tile([63, C], bf16, name="wc1a_t")
                nc.sync.dma_start(wc1a_t[:], wc1a[:])
                wc1b_t = p1pool.tile([63, C], bf16, name="wc1b_t")
                nc.sync.dma_start(wc1b_t[:], wc1b[:])
                wc1c_t = p1pool.tile([21, C], bf16, name="wc1c_t")
                nc.sync.dma_start(wc1c_t[:], wc1c[:])
                for th in range(NT):
                    r0 = th * 8
                    nc.tensor.matmul(y1r[:, r0:r0 + 8, :],
                                     wc1a_t[:], X1r[:, r0:r0 + 8, 0:64],
                                     start=True, stop=False)
                    nc.tensor.matmul(y1r[:, r0:r0 + 8, :],
                                     wc1b_t[:], X1r[:, 3 + r0:3 + r0 + 8, 0:64],
                                     start=False, stop=False)
                    nc.tensor.matmul(y1r[:, r0:r0 + 8, :],
                                     wc1c_t[0:21, :],
                                     X1r[0:21, 6 + r0:6 + r0 + 8, 0:64],
                                     start=False, stop=True)
                s1, t1 = _bn_block(nc, p1pool, dram, psum_y1[:],
                                   gb[:, 0:1], gb[:, 1:2], L, N_CORES * L, "bn1")
                x2r = x2pad[:].rearrange("p (r c) -> p r c", c=66)
                for th in range(NT):
                    nc.scalar.activation(
                        x2r[:, 1 + th * 8:1 + th * 8 + 8, 1:65],
                        y1r[:, th * 8:th * 8 + 8, :],
                        AF.Relu, bias=t1[:], scale=s1[:])

            # ---- Phase 2: conv2 + BN2 -> xb2 ----
            xb2 = cp_.tile([64, L], bf16, name="xb2")
            with tc.tile_pool(name="ph2", bufs=1) as p2pool, \
                 tc.tile_pool(name="ph2psum", bufs=1, space="PSUM") as psum2:
                psum_y2 = psum2.tile([C, L], f32, name="psum_y2")
                y2r = psum_y2[:].rearrange("p (r c) -> p r c", c=64)
                x2r = x2pad[:].rearrange("p (r c) -> p r c", c=66)
                for th in range(NT):
                    r0 = th * 8
                    first = True
                    for kh in range(3):
                        for kw in range(3):
                            nc.tensor.matmul(
                                y2r[:, r0:r0 + 8, :],
                                wc2_t[:, (kh * 3 + kw) * C:(kh * 3 + kw + 1) * C],
                                x2r[:, kh + r0:kh + r0 + 8, kw:kw + 64],
                                start=first, stop=(kh == 2 and kw == 2))
                            first = False
                s2, t2 = _bn_block(nc, p2pool, dram, psum_y2[:],
                                   gb[:, 2:3], gb[:, 3:4], L, N_CORES * L, "bn2")
                nc.scalar.activation(xb2[:], psum_y2[:], AF.Relu,
                                     bias=t2[:], scale=s2[:])

            # ---- Phase 3: conv3 (dup M=128) + BN3 -> X3s ----
            X3s = pp.tile([128, L + 4], bf16, name="X3s")
            nc.vector.memset(X3s[:], 0.0)
            with tc.tile_pool(name="ph3", bufs=1) as p3pool, \
                 tc.tile_pool(name="ph3psum", bufs=1, space="PSUM") as psum3:
                psum_y3 = psum3.tile([128, L], f32, name="psum_y3")
                for th in range(NT):
                    nc.tensor.matmul(psum_y3[:, th * TS:(th + 1) * TS],
                                     wc3_t[:], xb2[:, th * TS:(th + 1) * TS],
                                     start=True, stop=True)
                s3, t3 = _bn_block(nc, p3pool, dram, psum_y3[0:C, :],
                                   gb[:, 4:5], gb[:, 5:6], L, N_CORES * L, "bn3")
                nc.scalar.activation(X3s[0:64, 3:3 + L], psum_y3[0:64, :],
                                     AF.Relu, bias=t3[:], scale=s3[:])
                nc.scalar.activation(X3s[64:128, 2:2 + L], psum_y3[64:128, :],
                                     AF.Relu, bias=t3[:], scale=s3[:])
            cpool.__exit__(None, None, None)

            # ---- Phase 4: mamba projections ----
            xs_bf = pp.tile([DI, L], bf16, name="xs_bf")
            g_bf = pp.tile([DI, L], bf16, name="g_bf")
            dt_f = pp.tile([DI, L], f32, name="dt_f")
            u_bf = pp.tile([DI, L], bf16, name="u_bf")
            bc_bf = pp.tile([2 * DS, L], bf16, name="bc_bf")
            silu_insts = []
            with tc.tile_pool(name="ph4psum", bufs=2, space="PSUM") as psum4, \
                 tc.tile_pool(name="ph4", bufs=2) as p4pool:
                for th in range(NT):
                    sl = slice(th * TS, (th + 1) * TS)
                    psum_xc = psum4.tile([DI, TS], f32, tag="psum_xc")
                    nc.tensor.matmul(psum_xc[:], wina_t[:],
                                     X3s[:, th * TS:th * TS + TS],
                                     start=True, stop=False)
                    nc.tensor.matmul(psum_xc[:], winb_t[:],
                                     X3s[:, th * TS + 2:th * TS + 2 + TS],
                                     start=False, stop=True)
                    i1 = nc.scalar.activation(xs_bf[:, sl], psum_xc[:],
                                              AF.Silu, bias=convb_t[:])
                    psum_z = psum4.tile([DI, TS], f32, tag="psum_z")
                    nc.tensor.matmul(psum_z[:], wz_t[:],
                                     X3s[0:64, th * TS + 3:th * TS + 3 + TS],
                                     start=True, stop=True)
                    i2 = nc.scalar.activation(g_bf[:, sl], psum_z[:], AF.Silu)
                    silu_insts.append(i1)
                    silu_insts.append(i2)
                et_full = p4pool.tile([DI, L], f32, name="et_full")
                exp_insts = []
                for th in range(NT):
                    sl = slice(th * TS, (th + 1) * TS)
                    psum_dt = psum4.tile([DI, TS], f32, tag="psum_dt")
                    nc.tensor.matmul(psum_dt[:], wdt_t[:], xs_bf[:, sl],
                                     start=True, stop=True)
                    e1 = nc.scalar.activation(et_full[:, sl], psum_dt[:],
                                              AF.Exp, bias=dtb_t[:])
                    exp_insts.append(e1)
                    if th == 0:
                        for si in silu_insts:
                            tile.add_dep_helper(e1.ins, si.ins, sync=False,
                                                reason="act table grouping")
                    psum_bc = psum4.tile([2 * DS, TS], f32, tag="psum_bc")
                    nc.tensor.matmul(psum_bc[:], wbc_t[:], xs_bf[:, sl],
                                     start=True, stop=True)
                    nc.vector.tensor_copy(bc_bf[:, sl], psum_bc[:])
                for th in range(NT):
                    sl = slice(th * TS, (th + 1) * TS)
                    l1 = nc.scalar.activation(dt_f[:, sl], et_full[:, sl],
                                              AF.Ln, bias=1.0)
                    if th == 0:
                        for ei in exp_insts:
                            tile.add_dep_helper(l1.ins, ei.ins, sync=False,
                                                reason="act table grouping")
                    nc.vector.tensor_tensor(u_bf[:, sl], dt_f[:, sl],
                                            xs_bf[:, sl], OP.mult)

            # ---- Phase 5: selective scan, rotated s-assignment ----
            # rotation j: partition p handles state s = (p + j) % 16.
            # B/C replicas come from DRAM staging rows [j, j+128) where
            # staged row r holds B[r % 16].
            b3 = dram.tile([DI + DS - 1, L], bf16, name="b3")
            c3 = dram.tile([DI + DS - 1, L], bf16, name="c3")
            for r0 in range(0, DI + DS - 1, DS):
                n = min(DS, DI + DS - 1 - r0)
                nc.sync.dma_start(b3[r0:r0 + n, :], bc_bf[0:n, :])
                nc.scalar.dma_start(c3[r0:r0 + n, :], bc_bf[DS:DS + n, :])
            y0 = pp.tile([DI, L], bf16, name="y0")
            y1t = pp.tile([DI, L], bf16, name="y1t")
            with tc.tile_pool(name="ph5", bufs=2) as p5pool:
                for s in range(DS):
                    ball = p5pool.tile([DI, L], bf16, tag="ball", bufs=2)
                    nc.sync.dma_start(ball[:], b3[s:s + DI, :])
                    call = p5pool.tile([DI, L], bf16, tag="call", bufs=2)
                    nc.scalar.dma_start(call[:], c3[s:s + DI, :])
                    a_bf = p5pool.tile([DI, L], bf16, tag="a_bf", bufs=2)
                    nc.scalar.activation(a_bf[:], dt_f[:], AF.Exp,
                                         scale=acol_t[:, s:s + 1])
                    b_bf = p5pool.tile([DI, L], bf16, tag="b_bf", bufs=4)
                    h_bf = p5pool.tile([DI, L], bf16, tag="h_bf", bufs=2)
                    p_bf = p5pool.tile([DI, L], bf16, tag="p_bf", bufs=2)
                    nc.vector.tensor_tensor(b_bf[:], u_bf[:], ball[:], OP.mult)
                    nc.vector.tensor_tensor_scan(h_bf[:], a_bf[:], b_bf[:],
                                                 0.0, OP.mult, OP.add)
                    nc.vector.tensor_tensor(p_bf[:], h_bf[:], call[:], OP.mult)
                    yacc = y0 if s % 2 == 0 else y1t
                    if s < 2:
                        nc.vector.tensor_copy(yacc[:], p_bf[:])
                    else:
                        nc.vector.tensor_tensor(yacc[:], yacc[:], p_bf[:],
                                                OP.add)

            # ---- Phase 6: tail ----
            with tc.tile_pool(name="ph6", bufs=1) as p6pool:
                out_f = p6pool.tile([C, L], f32, name="out_f")
                ysum = p6pool.tile([DI, L], bf16, name="ysum")
                nc.vector.tensor_tensor(ysum[:], y0[:], y1t[:], OP.add)
                dx = p6pool.tile([DI, L], bf16, name="dx")
                nc.vector.tensor_scalar_mul(dx[:], xs_bf[:], dcol_t[:])
                y2m = p6pool.tile([DI, L], bf16, name="y2m")
                nc.vector.tensor_tensor(y2m[:], ysum[:], dx[:], OP.add)
                y3m = p6pool.tile([DI, L], bf16, name="y3m")
                nc.vector.tensor_tensor(y3m[:], y2m[:], g_bf[:], OP.mult)
                m_bf = p6pool.tile([C, L], bf16, name="m_bf")
                with tc.tile_pool(name="ph6psum", bufs=2,
                                  space="PSUM") as psum6:
                    for th in range(NT):
                        sl = slice(th * TS, (th + 1) * TS)
                        psum_m = psum6.tile([C, TS], f32, tag="psum_m")
                        nc.tensor.matmul(psum_m[:], wop_t[:], y3m[:, sl],
                                         start=True, stop=True)
                        nc.vector.tensor_copy(m_bf[:, sl], psum_m[:])
                with tc.tile_pool(name="ph6psum2", bufs=1,
                                  space="PSUM") as psum7:
                    psum_y4 = psum7.tile([C, L], f32, name="psum_y4")
                    for th in range(NT):
                        sl = slice(th * TS, (th + 1) * TS)
                        nc.tensor.matmul(psum_y4[:, sl], wc4_t[:], m_bf[:, sl],
                                         start=True, stop=False)
                        nc.tensor.matmul(psum_y4[:, sl], wc4_t[:],
                                         X3s[0:64, 3 + th * TS:3 + th * TS + TS],
                                         start=False, stop=True)
                    s4, t4 = _bn_block(nc, p6pool, dram, psum_y4[:],
                                       gb[:, 6:7], gb[:, 7:8], L,
                                       N_CORES * L, "bn4")
                    nc.scalar.activation(out_f[:], psum_y4[:], AF.Relu,
                                         bias=t4[:], scale=s4[:])
            nc.sync.dma_start(out_y[:], out_f[:])

    nc.compile()
    return nc


def _prep_weights(I):
    """Host-side weight layout prep (all tiny)."""
    bf = lambda a: np.ascontiguousarray(a, dtype=np.float32).astype(bfnp)
    f = lambda a: np.ascontiguousarray(a, dtype=np.float32)
    w1 = np.asarray(I["w1"], np.float32)      # (64, 3, 7, 7)
    t1 = w1.transpose(2, 3, 1, 0)             # (kh, kw, ci, co)
    # partitions (khl, kw, ci)
    wc1a = t1[0:3].transpose(0, 1, 2, 3).reshape(3, 7, 3, C)
    mk1 = lambda g: np.ascontiguousarray(
        t1[g * 3:g * 3 + 3].reshape(3, 7, 3, C)).reshape(63, C)
    wc1a = mk1(0)
    wc1b = mk1(1)
    wc1c = np.ascontiguousarray(t1[6:7].reshape(1, 7, 3, C)).reshape(21, C)
    w2 = np.asarray(I["w2"], np.float32)
    wc2 = w2.transpose(2, 3, 1, 0).reshape(9, C, C)  # (tap, ci, co)
    wc2 = np.ascontiguousarray(wc2.transpose(1, 0, 2)).reshape(C, 9 * C)
    w3 = np.asarray(I["w3"], np.float32)[:, :, 0, 0]  # (co, ci)
    wc3 = np.concatenate([w3.T, w3.T], axis=1)        # (ci, 2*co)
    in_proj = np.asarray(I["in_proj"], np.float32)    # (256, 64)
    Wxs, Wz = in_proj[:DI], in_proj[DI:]
    cw = np.asarray(I["conv_w"], np.float32)[:, 0, :]  # (di, 4)
    # wina[j*64+c, d] = cw[d, j] * Wxs[d, c], j in {0,1}
    wina = np.empty((128, DI), np.float32)
    winb = np.empty((128, DI), np.float32)
    for j in range(2):
        wina[j * 64:(j + 1) * 64] = (cw[:, j][:, None] * Wxs).T
        winb[j * 64:(j + 1) * 64] = (cw[:, 2 + j][:, None] * Wxs).T
    x_proj = np.asarray(I["x_proj"], np.float32)       # (36, 128)
    dt_pre = np.asarray(I["dt_w"], np.float32) @ x_proj[:R]  # (di, di)
    wdt = dt_pre.T
    wbc = x_proj[R:].T                                  # (128, 32)
    wop = np.asarray(I["out_proj"], np.float32).T       # (128, 64)
    w4 = np.asarray(I["w4"], np.float32)[:, :, 0, 0]
    wc4 = w4.T
    A = -np.exp(np.asarray(I["A_log"], np.float32))     # (di, ds)
    Arot = np.empty((DI, DS), np.float32)
    for j in range(DS):
        Arot[:, j] = A[np.arange(DI), (np.arange(DI) + j) % DS]
    gbcols = np.stack([f(I["g1"]), f(I["b1"]), f(I["g2"]), f(I["b2"]),
                       f(I["g3"]), f(I["b3"]), f(I["g4"]), f(I["b4"])],
                      axis=1)
    return {
        "wc1a": bf(wc1a), "wc1b": bf(wc1b), "wc1c": bf(wc1c),
        "wc2": bf(wc2), "wc3": bf(wc3),
        "wina": bf(wina), "winb": bf(winb), "wz": bf(Wz.T),
        "wdt": bf(wdt), "wbc": bf(wbc), "wop": bf(wop), "wc4": bf(wc4),
        "convb": f(I["conv_b"]).reshape(DI, 1),
        "dtb": f(I["dt_b"]).reshape(DI, 1),
        "acols": f(Arot), "dcol": f(I["D"]).reshape(DI, 1),
        "gbcols": f(gbcols),
    }


def kernel(**inputs):
    if "nc" not in _cache:
        _cache["nc"] = build_program()
    nc = _cache["nc"]
    wmap = _prep_weights(inputs)
    x = np.asarray(inputs["x"], np.float32)  # (8, 3, 128, 128)
    in_maps = []
    for b in range(N_CORES):
        m = dict(wmap)
        m["ximg"] = np.ascontiguousarray(x[b])
        in_maps.append(m)
    import os
    trace = bool(os.environ.get("KERNEL_TRACE"))
    if trace:
        try:
            import trace_shim  # noqa: F401  (dev-only profiling hook)
        except ImportError:
            trace = False
    res = bass_utils.run_bass_kernel_spmd(nc, in_maps,
                                          core_ids=list(range(N_CORES)),
                                          trace=trace)
    _cache["exec_time_ns"] = res.exec_time_ns
    out = np.empty((8, C, H, W), np.float32)
    for b in range(N_CORES):
        out[b] = res.results[b]["out_y"].reshape(C, H, W)
    lnb1 = np.asarray(inputs["lnb1"], np.float32).reshape(1, 1, 1, 1)
    lnb2 = np.asarray(inputs["lnb2"], np.float32).reshape(1, 1, 1, 1)
    p1 = np.broadcast_to(lnb1, (8, 1, H, W)).copy()
    p2 = np.broadcast_to(lnb2, (8, 1, H, W)).copy()
    return out, p1, p2


# revision 18
# speedup vs baseline: 1.3649x; 1.0684x over previous
"""Trainium2 Bass kernel for nn_Block_6012954214590.

Pipeline (per batch element, data-parallel over 8 NeuronCores):
  maxpool2x2 -> conv7x7+BN+ReLU -> conv3x3+BN+ReLU -> conv1x1+BN+ReLU
  -> Mamba block (in_proj [+folded depthwise causal conv], silu, x_proj,
     softplus dt, selective scan over L=4096 via DVE tensor_tensor_scan,
     gate, out_proj) -> residual -> conv1x1+BN+ReLU.
BN uses global batch statistics via tiny cross-core AllReduces (64x2 f32).
p1/p2 outputs are LayerNorm over a singleton channel axis == lnb exactly.
"""
import numpy as np
import ml_dtypes

import concourse.bass as bass
import concourse.bacc as bacc
import concourse.mybir as mybir
import concourse.tile as tile
from concourse import bass_utils

N_CORES = 8
C = 64          # conv channels
H = 64          # post-pool spatial
W = 64
L = H * W       # 4096
DI = 128        # mamba inner dim
DS = 16         # mamba state dim
R = 4           # dt rank
EPS = 1e-5
NT = 8          # N-tiles of 512 over L
TS = 512
f32 = mybir.dt.float32
bf16 = mybir.dt.bfloat16
AF = mybir.ActivationFunctionType
OP = mybir.AluOpType
bfnp = ml_dtypes.bfloat16

_cache = {}


def _bn_block(nc, pool, dram, psum_src, gcol, bcol, n_local, n_global, tag,
              parts=C):
    """Compute global-batch BN scale/bias from a psum tensor (parts, L).

    Returns (s_col, t_col) f32 (parts,1) tiles: out = relu(s*y + t).
    """
    C = parts  # noqa: N806 - reuse tile sizing below
    stats6 = pool.tile([C, 8 * 6], f32, name=f"stats6_{tag}")
    src3d = psum_src.rearrange("p (n f) -> p n f", f=TS)
    for i in range(8):
        nc.vector.bn_stats(stats6[:, i * 6:(i + 1) * 6], src3d[:, i, :])
    mv = pool.tile([C, 2], f32, name=f"mv_{tag}")
    nc.vector.bn_aggr(mv[:], stats6[:].rearrange("p (n s) -> p n s", s=6))
    # pack per-core (sum, sumsq)
    packed = pool.tile([C, 2], f32, name=f"packed_{tag}")
    nc.vector.tensor_scalar_mul(packed[:, 0:1], mv[:, 0:1], float(n_local))
    m2 = pool.tile([C, 1], f32, name=f"m2_{tag}")
    nc.vector.tensor_tensor(m2[:], mv[:, 0:1], mv[:, 0:1], OP.mult)
    vp = pool.tile([C, 1], f32, name=f"vp_{tag}")
    nc.vector.tensor_tensor(vp[:], mv[:, 1:2], m2[:], OP.add)
    nc.vector.tensor_scalar_mul(packed[:, 1:2], vp[:], float(n_local))
    # allreduce
    cin = dram.tile([C, 2], f32, name=f"arin_{tag}")
    cout = dram.tile([C, 2], f32, name=f"arout_{tag}")
    nc.sync.dma_start(cin[:], packed[:])
    nc.gpsimd.collective_compute(
        "AllReduce", OP.add, replica_groups=[list(range(N_CORES))],
        ins=[cin.opt()], outs=[cout.opt()],
    )
    glob = pool.tile([C, 2], f32, name=f"glob_{tag}")
    nc.sync.dma_start(glob[:], cout[:])
    # mu, var, rsqrt
    mu = pool.tile([C, 1], f32, name=f"mu_{tag}")
    nc.vector.tensor_scalar_mul(mu[:], glob[:, 0:1], 1.0 / n_global)
    e2 = pool.tile([C, 1], f32, name=f"e2_{tag}")
    nc.vector.tensor_scalar_mul(e2[:], glob[:, 1:2], 1.0 / n_global)
    m2g = pool.tile([C, 1], f32, name=f"m2g_{tag}")
    nc.vector.tensor_tensor(m2g[:], mu[:], mu[:], OP.mult)
    v = pool.tile([C, 1], f32, name=f"v_{tag}")
    nc.vector.tensor_tensor(v[:], e2[:], m2g[:], OP.subtract)
    nc.vector.tensor_scalar_add(v[:], v[:], EPS)
    # rsqrt via bit-hack seed + 3 Newton iterations (no ACT tables needed)
    i32 = mybir.dt.int32
    magic = pool.tile([C, 1], i32, name=f"magic_{tag}")
    nc.vector.memset(magic[:], 0x5f3759df)
    half = pool.tile([C, 1], i32, name=f"half_{tag}")
    nc.vector.tensor_scalar(half[:], v[:].bitcast(i32), 1, None,
                            OP.logical_shift_right)
    rs = pool.tile([C, 1], f32, name=f"rs_{tag}")
    nc.vector.tensor_tensor(rs[:].bitcast(i32), magic[:], half[:], OP.subtract)
    tmp = pool.tile([C, 1], f32, name=f"nt_{tag}")
    for _ in range(2):
        nc.vector.tensor_tensor(tmp[:], rs[:], rs[:], OP.mult)
        nc.vector.tensor_tensor(tmp[:], tmp[:], v[:], OP.mult)
        nc.vector.tensor_scalar(tmp[:], tmp[:], -0.5, 1.5, OP.mult, OP.add)
        nc.vector.tensor_tensor(rs[:], rs[:], tmp[:], OP.mult)
    s_col = pool.tile([C, 1], f32, name=f"scol_{tag}")
    nc.vector.tensor_tensor(s_col[:], rs[:], gcol, OP.mult)
    ms = pool.tile([C, 1], f32, name=f"ms_{tag}")
    nc.vector.tensor_tensor(ms[:], mu[:], s_col[:], OP.mult)
    t_col = pool.tile([C, 1], f32, name=f"tcol_{tag}")
    nc.vector.tensor_tensor(t_col[:], bcol, ms[:], OP.subtract)
    return s_col, t_col


def build_program():
    nc = bacc.Bacc("TRN2", target_bir_lowering=False, debug=False,
                   enable_asserts=False, num_devices=N_CORES)

    def din(name, shape, dt):
        return nc.dram_tensor(name, shape, dt, kind="ExternalInput").ap()

    ximg = din("ximg", [3, 128, 128], f32)
    wc1a = din("wc1a", [63, 2 * C], bf16)  # rows (khl, kw, ci), dup co
    wc1b = din("wc1b", [63, 2 * C], bf16)  # kh 3-5
    wc1c = din("wc1c", [21, 2 * C], bf16)  # kh 6
    wc2p = din("wc2p", [128, 3 * C], bf16)  # paired taps kh 0,1
    wc2s = din("wc2s", [64, 3 * C], bf16)   # single tap kh 2
    wc3 = din("wc3", [64, 2 * C], bf16)  # duplicated output channels
    wina = din("wina", [128, DI], bf16)  # (j*64+c, d) taps 0,1 of folded dwconv
    winb = din("winb", [128, DI], bf16)  # taps 2,3
    wz = din("wz", [64, DI], bf16)
    wdt = din("wdt", [DI, DI], bf16)     # dt_w @ x_proj[:4] transposed
    wbc = din("wbc", [DI, 2 * DS], bf16)
    wop = din("wop", [DI, C], bf16)
    wc4 = din("wc4", [64, C], bf16)
    convb = din("convb", [DI, 1], f32)
    dtb = din("dtb", [DI, 1], f32)
    acols = din("acols", [DI, DS], f32)  # A = -exp(A_log)
    dcol = din("dcol", [DI, 1], f32)
    gbcols = din("gbcols", [2 * C, 8], f32)  # g1 b1 .. g4 b4 (dup rows)
    out_y = nc.dram_tensor("out_y", [C, L], f32, kind="ExternalOutput").ap()

    with tile.TileContext(nc) as tc:
        with tc.tile_pool(name="persist", bufs=1) as pp, \
             tc.tile_pool(name="dram", bufs=1, space="DRAM") as dram:
            gb = pp.tile([2 * C, 8], f32, name="gb")
            nc.sync.dma_start(gb[:], gbcols[:])
            acol_t = pp.tile([DI, DS], f32, name="acol_t")
            nc.sync.dma_start(acol_t[:], acols[:])
            convb_t = pp.tile([DI, 1], f32, name="convb_t")
            nc.sync.dma_start(convb_t[:], convb[:])
            dtb_t = pp.tile([DI, 1], f32, name="dtb_t")
            nc.sync.dma_start(dtb_t[:], dtb[:])
            dcol_t = pp.tile([DI, 1], f32, name="dcol_t")
            nc.sync.dma_start(dcol_t[:], dcol[:])
            ones1 = pp.tile([65, 128], bf16, name="ones1")
            nc.vector.memset(ones1[:], 1.0)
            warm = pp.tile([C, 2], f32, name="warm")
            nc.vector.memset(warm[:], 0.0)
            win_d = dram.tile([C, 2], f32, name="warm_in")
            wout_d = dram.tile([C, 2], f32, name="warm_out")
            nc.sync.dma_start(win_d[:], warm[:])
            nc.gpsimd.collective_compute(
                "AllReduce", OP.add, replica_groups=[list(range(N_CORES))],
                ins=[win_d.opt()], outs=[wout_d.opt()],
            )
            # weights
            wc2p_t = pp.tile([128, 3 * C], bf16, name="wc2p_t")
            nc.sync.dma_start(wc2p_t[:], wc2p[:])
            wc2s_t = pp.tile([64, 3 * C], bf16, name="wc2s_t")
            nc.sync.dma_start(wc2s_t[:], wc2s[:])
            wc3_t = pp.tile([64, 2 * C], bf16, name="wc3_t")
            nc.sync.dma_start(wc3_t[:], wc3[:])
            wina_t = pp.tile([128, DI], bf16, name="wina_t")
            nc.sync.dma_start(wina_t[:], wina[:])
            winb_t = pp.tile([128, DI], bf16, name="winb_t")
            nc.sync.dma_start(winb_t[:], winb[:])
            wz_t = pp.tile([64, DI], bf16, name="wz_t")
            nc.sync.dma_start(wz_t[:], wz[:])
            wdt_t = pp.tile([DI, DI], bf16, name="wdt_t")
            nc.sync.dma_start(wdt_t[:], wdt[:])
            wbc_t = pp.tile([DI, 2 * DS], bf16, name="wbc_t")
            nc.sync.dma_start(wbc_t[:], wbc[:])
            wop_t = pp.tile([DI, C], bf16, name="wop_t")
            nc.sync.dma_start(wop_t[:], wop[:])
            wc4_t = pp.tile([64, C], bf16, name="wc4_t")
            nc.sync.dma_start(wc4_t[:], wc4[:])

            # ---- Phase 1: maxpool + conv1 + BN1 -> x2pad ----
            cpool = tc.tile_pool(name="convs", bufs=1)
            cp_ = cpool.__enter__()
            x2pad = cp_.tile([128, 66 * 66], bf16, name="x2pad")
            nc.vector.memset(x2pad[:], 0.0)
            with tc.tile_pool(name="ph1", bufs=1) as p1pool, \
                 tc.tile_pool(name="ph1psum", bufs=1, space="PSUM") as psum1:
                T = p1pool.tile([64, 768], f32, name="T")
                nc.sync.dma_start(
                    T[:].rearrange("p (c hp w) -> p c hp w", c=3, hp=2),
                    ximg.rearrange("c (p hp) w -> p c hp w", hp=2))
                T4 = T[:].rearrange("p (c hp w) -> p c hp w", c=3, hp=2)
                P2 = p1pool.tile([64, 384], f32, name="P2")
                P24 = P2[:].rearrange("p (c hp w) -> p c hp w", c=3, hp=2)
                nc.vector.tensor_tensor(P24, T4[:, :, :, 0:128:2],
                                        T4[:, :, :, 1:128:2], OP.max)
                Pool = p1pool.tile([64, 192], f32, name="Pool")
                P23 = P2[:].rearrange("p (c hp w) -> p c hp w", c=3, hp=2)
                Pl3 = Pool[:].rearrange("p (c w) -> p c w", c=3)
                nc.vector.tensor_tensor(Pl3, P23[:, :, 0, :], P23[:, :, 1, :],
                                        OP.max)
                # stage pooled to DRAM as a zero-padded flat image (3, 5120)
                zpad = p1pool.tile([3, 5120], bf16, name="zpad")
                nc.vector.memset(zpad[:], 0.0)
                dpad = dram.tile([3, 5120], bf16, name="dpad")
                zfill = nc.sync.dma_start(dpad[:], zpad[:])
                dpad3 = dpad[:, 0:4900].rearrange("c (r w) -> c r w", w=70)
                ifill = nc.gpsimd.dma_start(
                    dpad3[:, 3:67, 3:67].rearrange("c r w -> r c w"), Pl3)
                tile.add_dep_helper(ifill.ins, zfill.ins,
                                    reason="pad zeros before interior")
                # X1: partitions (khl, kw, ci) = xpadflat shifted khl*70 + kw
                X1 = p1pool.tile([63, 4900], bf16, name="X1")
                qi = 0
                for khl in range(3):
                    for kw in range(7):
                        p0 = khl * 21 + kw * 3
                        o = khl * 70 + kw
                        eng = (nc.sync, nc.scalar)[qi % 2]
                        eng.dma_start(X1[p0:p0 + 3, :], dpad[:, o:o + 4900])
                        qi += 1
                psum_y1 = psum1.tile([2 * C, L], f32, name="psum_y1")
                X1r = X1[:, 0:4900].rearrange("p (r c) -> p r c", c=70)
                y1r = psum_y1[:].rearrange("p (r c) -> p r c", c=64)
                wc1a_t = p1pool.tile([63, 2 * C], bf16, name="wc1a_t")
                nc.sync.dma_start(wc1a_t[:], wc1a[:])
                wc1b_t = p1pool.tile([63, 2 * C], bf16, name="wc1b_t")
                nc.sync.dma_start(wc1b_t[:], wc1b[:])
                wc1c_t = p1pool.tile([21, 2 * C], bf16, name="wc1c_t")
                nc.sync.dma_start(wc1c_t[:], wc1c[:])
                for th in range(NT):
                    r0 = th * 8
                    nc.tensor.matmul(y1r[:, r0:r0 + 8, :],
                                     wc1a_t[:], X1r[:, r0:r0 + 8, 0:64],
                                     start=True, stop=False)
                    nc.tensor.matmul(y1r[:, r0:r0 + 8, :],
                                     wc1b_t[:], X1r[:, 3 + r0:3 + r0 + 8, 0:64],
                                     start=False, stop=False)
                    nc.tensor.matmul(y1r[:, r0:r0 + 8, :],
                                     wc1c_t[0:21, :],
                                     X1r[0:21, 6 + r0:6 + r0 + 8, 0:64],
                                     start=False, stop=True)
                s1, t1 = _bn_block(nc, p1pool, dram, psum_y1[:],
                                   gb[:, 0:1], gb[:, 1:2], L, N_CORES * L,
                                   "bn1", parts=2 * C)
                x2r = x2pad[:].rearrange("p (r c) -> p r c", c=66)
                for th in range(NT):
                    nc.scalar.activation(
                        x2r[0:64, 1 + th * 8:1 + th * 8 + 8, 1:65],
                        y1r[0:64, th * 8:th * 8 + 8, :],
                        AF.Relu, bias=t1[0:64, :], scale=s1[0:64, :])
                    nc.scalar.activation(
                        x2r[64:128, th * 8:th * 8 + 8, 1:65],
                        y1r[64:128, th * 8:th * 8 + 8, :],
                        AF.Relu, bias=t1[64:128, :], scale=s1[64:128, :])

            # ---- Phase 2: conv2 + BN2 -> xb2 ----
            xb2 = cp_.tile([64, L], bf16, name="xb2")
            with tc.tile_pool(name="ph2", bufs=1) as p2pool, \
                 tc.tile_pool(name="ph2psum", bufs=1, space="PSUM") as psum2:
                psum_y2 = psum2.tile([C, L], f32, name="psum_y2")
                y2r = psum_y2[:].rearrange("p (r c) -> p r c", c=64)
                x2r = x2pad[:].rearrange("p (r c) -> p r c", c=66)
                for th in range(NT):
                    r0 = th * 8
                    for kw in range(3):
                        nc.tensor.matmul(
                            y2r[:, r0:r0 + 8, :],
                            wc2p_t[:, kw * C:(kw + 1) * C],
                            x2r[:, r0:r0 + 8, kw:kw + 64],
                            start=(kw == 0), stop=False)
                    for kw in range(3):
                        nc.tensor.matmul(
                            y2r[:, r0:r0 + 8, :],
                            wc2s_t[:, kw * C:(kw + 1) * C],
                            x2r[0:64, 2 + r0:2 + r0 + 8, kw:kw + 64],
                            start=False, stop=(kw == 2))
                s2, t2 = _bn_block(nc, p2pool, dram, psum_y2[:],
                                   gb[0:C, 2:3], gb[0:C, 3:4], L, N_CORES * L, "bn2")
                nc.scalar.activation(xb2[:], psum_y2[:], AF.Relu,
                                     bias=t2[:], scale=s2[:])

            # ---- Phase 3: conv3 (dup M=128) + BN3 -> X3s ----
            X3s = pp.tile([128, L + 4], bf16, name="X3s")
            nc.vector.memset(X3s[:], 0.0)
            with tc.tile_pool(name="ph3", bufs=1) as p3pool, \
                 tc.tile_pool(name="ph3psum", bufs=1, space="PSUM") as psum3:
                psum_y3 = psum3.tile([128, L], f32, name="psum_y3")
                for th in range(NT):
                    nc.tensor.matmul(psum_y3[:, th * TS:(th + 1) * TS],
                                     wc3_t[:], xb2[:, th * TS:(th + 1) * TS],
                                     start=True, stop=True)
                s3, t3 = _bn_block(nc, p3pool, dram, psum_y3[0:C, :],
                                   gb[0:C, 4:5], gb[0:C, 5:6], L, N_CORES * L, "bn3")
                nc.scalar.activation(X3s[0:64, 3:3 + L], psum_y3[0:64, :],
                                     AF.Relu, bias=t3[:], scale=s3[:])
                nc.scalar.activation(X3s[64:128, 2:2 + L], psum_y3[64:128, :],
                                     AF.Relu, bias=t3[:], scale=s3[:])
            cpool.__exit__(None, None, None)

            # ---- Phase 4: mamba projections ----
            xs_bf = pp.tile([DI, L], bf16, name="xs_bf")
            g_bf = pp.tile([DI, L], bf16, name="g_bf")
            dt_f = pp.tile([DI, L], f32, name="dt_f")
            u_bf = pp.tile([DI, L], bf16, name="u_bf")
            bc_bf = pp.tile([2 * DS, L], bf16, name="bc_bf")
            silu_insts = []
            with tc.tile_pool(name="ph4psum", bufs=2, space="PSUM") as psum4, \
                 tc.tile_pool(name="ph4", bufs=2) as p4pool:
                for th in range(NT):
                    sl = slice(th * TS, (th + 1) * TS)
                    psum_xc = psum4.tile([DI, TS], f32, tag="psum_xc")
                    nc.tensor.matmul(psum_xc[:], wina_t[:],
                                     X3s[:, th * TS:th * TS + TS],
                                     start=True, stop=False)
                    nc.tensor.matmul(psum_xc[:], winb_t[:],
                                     X3s[:, th * TS + 2:th * TS + 2 + TS],
                                     start=False, stop=True)
                    i1 = nc.scalar.activation(xs_bf[:, sl], psum_xc[:],
                                              AF.Silu, bias=convb_t[:])
                    psum_z = psum4.tile([DI, TS], f32, tag="psum_z")
                    nc.tensor.matmul(psum_z[:], wz_t[:],
                                     X3s[0:64, th * TS + 3:th * TS + 3 + TS],
                                     start=True, stop=True)
                    i2 = nc.scalar.activation(g_bf[:, sl], psum_z[:], AF.Silu)
                    silu_insts.append(i1)
                    silu_insts.append(i2)
                et_full = p4pool.tile([DI, L], f32, name="et_full")
                exp_insts = []
                for th in range(NT):
                    sl = slice(th * TS, (th + 1) * TS)
                    psum_dt = psum4.tile([DI, TS], f32, tag="psum_dt")
                    nc.tensor.matmul(psum_dt[:], wdt_t[:], xs_bf[:, sl],
                                     start=True, stop=True)
                    e1 = nc.scalar.activation(et_full[:, sl], psum_dt[:],
                                              AF.Exp, bias=dtb_t[:])
                    exp_insts.append(e1)
                    if th == 0:
                        for si in silu_insts:
                            tile.add_dep_helper(e1.ins, si.ins, sync=False,
                                                reason="act table grouping")
                    psum_bc = psum4.tile([2 * DS, TS], f32, tag="psum_bc")
                    nc.tensor.matmul(psum_bc[:], wbc_t[:], xs_bf[:, sl],
                                     start=True, stop=True)
                    nc.vector.tensor_copy(bc_bf[:, sl], psum_bc[:])
                for th in range(NT):
                    sl = slice(th * TS, (th + 1) * TS)
                    l1 = nc.scalar.activation(dt_f[:, sl], et_full[:, sl],
                                              AF.Ln, bias=1.0)
                    if th == 0:
                        for ei in exp_insts:
                            tile.add_dep_helper(l1.ins, ei.ins, sync=False,
                                                reason="act table grouping")
                    nc.vector.tensor_tensor(u_bf[:, sl], dt_f[:, sl],
                                            xs_bf[:, sl], OP.mult)

            # ---- Phase 5: selective scan, rotated s-assignment ----
            # rotation j: partition p handles state s = (p + j) % 16.
            # B/C replicas come from DRAM staging rows [j, j+128) where
            # staged row r holds B[r % 16].
            b3 = dram.tile([DI + DS - 1, L], bf16, name="b3")
            c3 = dram.tile([DI + DS - 1, L], bf16, name="c3")
            for r0 in range(0, DI + DS - 1, DS):
                n = min(DS, DI + DS - 1 - r0)
                nc.sync.dma_start(b3[r0:r0 + n, :], bc_bf[0:n, :])
                nc.scalar.dma_start(c3[r0:r0 + n, :], bc_bf[DS:DS + n, :])
            y0 = pp.tile([DI, L], bf16, name="y0")
            y1t = pp.tile([DI, L], bf16, name="y1t")
            with tc.tile_pool(name="ph5", bufs=2) as p5pool:
                for s in range(DS):
                    ball = p5pool.tile([DI, L], bf16, tag="ball", bufs=2)
                    nc.sync.dma_start(ball[:], b3[s:s + DI, :])
                    call = p5pool.tile([DI, L], bf16, tag="call", bufs=2)
                    nc.scalar.dma_start(call[:], c3[s:s + DI, :])
                    a_bf = p5pool.tile([DI, L], bf16, tag="a_bf", bufs=2)
                    nc.scalar.activation(a_bf[:], dt_f[:], AF.Exp,
                                         scale=acol_t[:, s:s + 1])
                    b_bf = p5pool.tile([DI, L], bf16, tag="b_bf", bufs=4)
                    h_bf = p5pool.tile([DI, L], bf16, tag="h_bf", bufs=2)
                    p_bf = p5pool.tile([DI, L], bf16, tag="p_bf", bufs=2)
                    nc.vector.tensor_tensor(b_bf[:], u_bf[:], ball[:], OP.mult)
                    nc.vector.tensor_tensor_scan(h_bf[:], a_bf[:], b_bf[:],
                                                 0.0, OP.mult, OP.add)
                    nc.vector.tensor_tensor(p_bf[:], h_bf[:], call[:], OP.mult)
                    yacc = y0 if s % 2 == 0 else y1t
                    if s < 2:
                        nc.vector.tensor_copy(yacc[:], p_bf[:])
                    else:
                        nc.vector.tensor_tensor(yacc[:], yacc[:], p_bf[:],
                                                OP.add)

            # ---- Phase 6: tail ----
            with tc.tile_pool(name="ph6", bufs=1) as p6pool:
                out_f = p6pool.tile([C, L], f32, name="out_f")
                ysum = p6pool.tile([DI, L], bf16, name="ysum")
                nc.vector.tensor_tensor(ysum[:], y0[:], y1t[:], OP.add)
                dx = p6pool.tile([DI, L], bf16, name="dx")
                nc.vector.tensor_scalar_mul(dx[:], xs_bf[:], dcol_t[:])
                y2m = p6pool.tile([DI, L], bf16, name="y2m")
                nc.vector.tensor_tensor(y2m[:], ysum[:], dx[:], OP.add)
                y3m = p6pool.tile([DI, L], bf16, name="y3m")
                nc.vector.tensor_tensor(y3m[:], y2m[:], g_bf[:], OP.mult)
                m_bf = p6pool.tile([C, L], bf16, name="m_bf")
                with tc.tile_pool(name="ph6psum", bufs=2,
                                  space="PSUM") as psum6:
                    for th in range(NT):
                        sl = slice(th * TS, (th + 1) * TS)
                        psum_m = psum6.tile([C, TS], f32, tag="psum_m")
                        nc.tensor.matmul(psum_m[:], wop_t[:], y3m[:, sl],
                                         start=True, stop=True)
                        nc.vector.tensor_copy(m_bf[:, sl], psum_m[:])
                with tc.tile_pool(name="ph6psum2", bufs=1,
                                  space="PSUM") as psum7:
                    psum_y4 = psum7.tile([C, L], f32, name="psum_y4")
                    for th in range(NT):
                        sl = slice(th * TS, (th + 1) * TS)
                        nc.tensor.matmul(psum_y4[:, sl], wc4_t[:], m_bf[:, sl],
                                         start=True, stop=False)
                        nc.tensor.matmul(psum_y4[:, sl], wc4_t[:],
                                         X3s[0:64, 3 + th * TS:3 + th * TS + TS],
                                         start=False, stop=True)
                    s4, t4 = _bn_block(nc, p6pool, dram, psum_y4[:],
                                       gb[0:C, 6:7], gb[0:C, 7:8], L,
                                       N_CORES * L, "bn4")
                    nc.scalar.activation(out_f[:], psum_y4[:], AF.Relu,
                                         bias=t4[:], scale=s4[:])
            nc.sync.dma_start(out_y[:], out_f[:])

    nc.compile()
    return nc


def _prep_weights(I):
    """Host-side weight layout prep (all tiny)."""
    bf = lambda a: np.ascontiguousarray(a, dtype=np.float32).astype(bfnp)
    f = lambda a: np.ascontiguousarray(a, dtype=np.float32)
    w1 = np.asarray(I["w1"], np.float32)      # (64, 3, 7, 7)
    t1 = w1.transpose(2, 3, 1, 0)             # (kh, kw, ci, co)
    # partitions (khl, kw, ci)
    wc1a = t1[0:3].transpose(0, 1, 2, 3).reshape(3, 7, 3, C)
    mk1 = lambda g: np.ascontiguousarray(
        t1[g * 3:g * 3 + 3].reshape(3, 7, 3, C)).reshape(63, C)
    wc1a = np.tile(mk1(0), (1, 2))
    wc1b = np.tile(mk1(1), (1, 2))
    wc1c = np.tile(
        np.ascontiguousarray(t1[6:7].reshape(1, 7, 3, C)).reshape(21, C),
        (1, 2))
    w2 = np.asarray(I["w2"], np.float32)
    # wc2p[64*j + ci, kw*64 + co] = w2[co, ci, j, kw]
    wc2p = np.ascontiguousarray(
        w2.transpose(2, 1, 3, 0)[0:2]).reshape(128, 3 * C)
    wc2s = np.ascontiguousarray(
        w2.transpose(2, 1, 3, 0)[2]).reshape(64, 3 * C)
    w3 = np.asarray(I["w3"], np.float32)[:, :, 0, 0]  # (co, ci)
    wc3 = np.concatenate([w3.T, w3.T], axis=1)        # (ci, 2*co)
    in_proj = np.asarray(I["in_proj"], np.float32)    # (256, 64)
    Wxs, Wz = in_proj[:DI], in_proj[DI:]
    cw = np.asarray(I["conv_w"], np.float32)[:, 0, :]  # (di, 4)
    # wina[j*64+c, d] = cw[d, j] * Wxs[d, c], j in {0,1}
    wina = np.empty((128, DI), np.float32)
    winb = np.empty((128, DI), np.float32)
    for j in range(2):
        wina[j * 64:(j + 1) * 64] = (cw[:, j][:, None] * Wxs).T
        winb[j * 64:(j + 1) * 64] = (cw[:, 2 + j][:, None] * Wxs).T
    x_proj = np.asarray(I["x_proj"], np.float32)       # (36, 128)
    dt_pre = np.asarray(I["dt_w"], np.float32) @ x_proj[:R]  # (di, di)
    wdt = dt_pre.T
    wbc = x_proj[R:].T                                  # (128, 32)
    wop = np.asarray(I["out_proj"], np.float32).T       # (128, 64)
    w4 = np.asarray(I["w4"], np.float32)[:, :, 0, 0]
    wc4 = w4.T
    A = -np.exp(np.asarray(I["A_log"], np.float32))     # (di, ds)
    Arot = np.empty((DI, DS), np.float32)
    for j in range(DS):
        Arot[:, j] = A[np.arange(DI), (np.arange(DI) + j) % DS]
    gbcols = np.stack([f(I["g1"]), f(I["b1"]), f(I["g2"]), f(I["b2"]),
                       f(I["g3"]), f(I["b3"]), f(I["g4"]), f(I["b4"])],
                      axis=1)
    gbcols = np.concatenate([gbcols, gbcols], axis=0)
    return {
        "wc1a": bf(wc1a), "wc1b": bf(wc1b), "wc1c": bf(wc1c),
        "wc2p": bf(wc2p), "wc2s": bf(wc2s), "wc3": bf(wc3),
        "wina": bf(wina), "winb": bf(winb), "wz": bf(Wz.T),
        "wdt": bf(wdt), "wbc": bf(wbc), "wop": bf(wop), "wc4": bf(wc4),
        "convb": f(I["conv_b"]).reshape(DI, 1),
        "dtb": f(I["dt_b"]).reshape(DI, 1),
        "acols": f(Arot), "dcol": f(I["D"]).reshape(DI, 1),
        "gbcols": f(gbcols),
    }


def kernel(**inputs):
    if "nc" not in _cache:
        _cache["nc"] = build_program()
    nc = _cache["nc"]
    wmap = _prep_weights(inputs)
    x = np.asarray(inputs["x"], np.float32)  # (8, 3, 128, 128)
    in_maps = []
    for b in range(N_CORES):
        m = dict(wmap)
        m["ximg"] = np.ascontiguousarray(x[b])
        in_maps.append(m)
    import os
    trace = bool(os.environ.get("KERNEL_TRACE"))
    if trace:
        try:
            import trace_shim  # noqa: F401  (dev-only profiling hook)
        except ImportError:
            trace = False
    res = bass_utils.run_bass_kernel_spmd(nc, in_maps,
                                          core_ids=list(range(N_CORES)),
                                          trace=trace)
    _cache["exec_time_ns"] = res.exec_time_ns
    out = np.empty((8, C, H, W), np.float32)
    for b in range(N_CORES):
        out[b] = res.results[b]["out_y"].reshape(C, H, W)
    lnb1 = np.asarray(inputs["lnb1"], np.float32).reshape(1, 1, 1, 1)
    lnb2 = np.asarray(inputs["lnb2"], np.float32).reshape(1, 1, 1, 1)
    p1 = np.broadcast_to(lnb1, (8, 1, H, W)).copy()
    p2 = np.broadcast_to(lnb2, (8, 1, H, W)).copy()
    return out, p1, p2
